# revision 1
# baseline (speedup 1.0000x reference)
"""Two-layer GRU (16->128->128) + FC(128->24) head on 8 Trainium2 NeuronCores.

Strategy: data-parallel over the batch (4096 -> 512 per core); tiny weights
replicated. On each core the hidden state lives transposed in SBUF as
[H=128 partitions, B=512 free]. Per time step, gate pre-activations are
accumulated in PSUM by fp32r matmuls (input-projection + recurrent + biases
folded in), sigmoid/tanh run on the scalar engine with per-partition bias
APs, and the cell update is spread across vector + gpsimd engines.

Self-contained: hardcodes all shapes; host-side prep only reshapes/transposes
numpy arrays (sharding + time-major packing of x, weight transposes).
"""

import numpy as np

import bass_rust
import concourse.bass as bass
import concourse.mybir as mybir
from concourse.tile import TileContext
from concourse.vector_clock import ScopedClock
from concourse.bass_utils import run_bass_kernel_spmd

N_CORES = 8
B_TOT = 4096
L = 128          # sequence length (= 2*1024/16)
D = 16           # per-step input features
DA = 17          # + ones row (bias folding for layer 1)
H = 128          # hidden
G3 = 3 * H       # 384 stacked gates (r, z, n)
BL = B_TOT // N_CORES  # 512 batch per core
NCLS = 24
CHUNK = 8        # time steps of x staged into SBUF per DMA

F32 = mybir.dt.float32
F32R = mybir.dt.float32r
F16 = mybir.dt.float16
BF16 = mybir.dt.bfloat16
AF = mybir.ActivationFunctionType
OP = mybir.AluOpType

# Tunables (grid-searched via TimelineSim, validated on HW).
CONFIG = {
    "dtype": "f16",      # gate/h/weight/x dtype: "f32r" | "f16" | "bf16"
    "pre_n_pe": True,    # accumulate t2 into P_x via PE identity matmul
    "split_rz1": True,  # separate r/z sigmoids for layer 1
    "d_eng": "v",        # engine for d = h - n
    "e_eng": "v",        # engine for e = z * d
    "h_eng": "v",        # engine for h' = n + e
}

_DT = {"f32r": F32R, "f16": F16, "bf16": BF16}
_NP_DT = {"f32r": np.float32, "f16": np.float16}


class SplitDrainTileContext(TileContext):
    """Walrus (CoreV3) rejects instructions carrying >2 sync waits; Tile's
    kernel-tail drain accumulates one wait per outstanding engine/DMA-queue
    sem. Split them across a chain of drains (1 wait each)."""

    def _drain_and_barrier(self, tick_clock, wait_clock):
        nc = self.nc
        drain_inst = nc.sync.drain()
        wait_clock.add_sem_waits(
            drain_inst.ins, ScopedClock({None: tick_clock.global_clock})
        )
        si = drain_inst.ins.sync_info
        if si is not None and len(si.on_wait) > 1:
            waits = list(si.on_wait)
            si.on_wait = waits[:1]
            for w in waits[1:]:
                d2 = nc.sync.drain()
                d2.ins.sync_info = bass_rust.SyncInfo(on_wait=[w], on_update=[])
        nc.all_engine_barrier()
        popped = nc._tile_sem_poison_stack.pop()
        assert popped is self._sem_poison
        nc.clear_and_free_semaphores(list(self.sems.allocated().values()))
        nc.all_engine_barrier()


def _split_excess_waits(nc: bass.Bass, max_waits: int = 1) -> None:
    """Walrus (CoreV3 setupSyncWait) accepts at most 2 sem waits per
    instruction; Tile occasionally attaches 3+. Hoist the excess onto
    EventSemaphore instructions inserted right before the offender on the
    same engine (serial waits AND together)."""
    n = 0
    for fn in nc.m.functions:
        for bb in fn.blocks:
            out = []
            dirty = False
            for inst in bb.instructions:
                si = inst.sync_info
                if si is not None and len(si.on_wait) > max_waits:
                    waits = list(si.on_wait)
                    extra = waits[: len(waits) - max_waits]
                    for w in extra:
                        ev = mybir.InstEventSemaphore(
                            name=f"evs-waitsplit-{n}", ins=[], outs=[]
                        )
                        n += 1
                        ev.engine = inst.engine
                        ev.sync_info = bass_rust.SyncInfo(
                            on_wait=[w], on_update=[]
                        )
                        out.append(ev)
                    si.on_wait = waits[len(waits) - max_waits :]
                    dirty = True
                out.append(inst)
            if dirty:
                bb.instructions = out


def build_program(for_sim: bool = False, n_steps: int = L) -> bass.Bass:
    # for_sim: skip the walrus wait-limit workarounds (post-hoc IR mutations
    # that CoreSim's bookkeeping doesn't understand); semantics identical.
    nc = bass.Bass()

    # Per-core DRAM I/O. Matmul operands are declared float32r (same bytes as
    # fp32) so the PE runs them at 1 cycle/row instead of fp32's 4.
    DT = _DT[CONFIG["dtype"]]
    xT_d = nc.declare_dram_parameter("xT", [L, DA, BL], DT, isOutput=False)
    l1w_d = nc.declare_dram_parameter("l1w", [DA, G3], DT, isOutput=False)
    hh1_d = nc.declare_dram_parameter("hh1w", [H, G3], DT, isOutput=False)
    ih2_d = nc.declare_dram_parameter("ih2w", [H, G3], DT, isOutput=False)
    hh2_d = nc.declare_dram_parameter("hh2w", [H, G3], DT, isOutput=False)
    bias_d = nc.declare_dram_parameter("bvec", [H, 5], F32, isOutput=False)
    fcw_d = nc.declare_dram_parameter("fcw", [H, NCLS], DT, isOutput=False)
    fcb_d = nc.declare_dram_parameter("fcb", [NCLS, 1], F32, isOutput=False)
    ident_d = nc.declare_dram_parameter("ident", [H, H], DT, isOutput=False)
    out_d = nc.declare_dram_parameter("outT", [NCLS, BL], F32, isOutput=True)

    tc_cls = TileContext if for_sim else SplitDrainTileContext
    with tc_cls(nc) as tc:
        with (
            tc.tile_pool(name="singles", bufs=1) as singles,
            tc.tile_pool(name="xchunks", bufs=3) as xpool,
            tc.tile_pool(name="hstate", bufs=2) as hpool,
            tc.tile_pool(name="work", bufs=3) as work,
            tc.tile_pool(name="prz", bufs=1, space="PSUM") as przpool,
            tc.tile_pool(name="pnx", bufs=1, space="PSUM") as pnxpool,
        ):
            # --- constant loads -------------------------------------------
            l1w = singles.tile([DA, G3], DT, tag="l1w")
            hh1w = singles.tile([H, G3], DT, tag="hh1w")
            ih2w = singles.tile([H, G3], DT, tag="ih2w")
            hh2w = singles.tile([H, G3], DT, tag="hh2w")
            sbias = singles.tile([H, 5], F32, tag="sbias")
            fcw = singles.tile([H, NCLS], DT, tag="fcw")
            fcb = singles.tile([NCLS, 1], F32, tag="fcb")
            ident = singles.tile([H, H], DT, tag="ident")
            nc.sync.dma_start(out=ident[:], in_=ident_d[:])
            nc.sync.dma_start(out=l1w[:], in_=l1w_d[:])
            nc.sync.dma_start(out=hh1w[:], in_=hh1_d[:])
            nc.sync.dma_start(out=ih2w[:], in_=ih2_d[:])
            nc.sync.dma_start(out=hh2w[:], in_=hh2_d[:])
            nc.sync.dma_start(out=sbias[:], in_=bias_d[:])
            nc.sync.dma_start(out=fcw[:], in_=fcw_d[:])
            nc.sync.dma_start(out=fcb[:], in_=fcb_d[:])

            ENG = {"v": nc.vector, "g": nc.gpsimd}

            def cell(tag, h_prev, x_rhs, xw, hw, rz_bias, n_hh_bias, n_ih_bias):
                """One GRU cell step, transposed layout [H partitions, BL free].

                h_prev: [H, BL] DT tile or None (t=0 => h=0, recurrent
                matmuls skipped). x_rhs: [K, BL] DT rhs for the input
                projection with lhsT xw [K, G3]; hw: [H, G3] recurrent lhsT.
                rz_bias: None (folded into xw) or (r_bias_ap, z_bias_ap).
                Returns the new [H, BL] DT hidden tile.
                """
                prz = przpool.tile([H, 2 * BL], F32, tag=f"prz{tag}")
                pn = pnxpool.tile([H, BL], F32, tag=f"pn{tag}")
                px = pnxpool.tile([H, BL], F32, tag=f"px{tag}")
                nc.tensor.matmul(prz[:, 0:BL], xw[:, 0:H], x_rhs,
                                 start=True, stop=h_prev is None)
                nc.tensor.matmul(prz[:, BL:], xw[:, H : 2 * H], x_rhs,
                                 start=True, stop=h_prev is None)
                if h_prev is not None:
                    nc.tensor.matmul(prz[:, 0:BL], hw[:, 0:H], h_prev[:],
                                     start=False, stop=True)
                    nc.tensor.matmul(prz[:, BL:], hw[:, H : 2 * H], h_prev[:],
                                     start=False, stop=True)
                nc.tensor.matmul(px[:], xw[:, 2 * H :], x_rhs, start=True,
                                 stop=not CONFIG["pre_n_pe"])
                if h_prev is not None:
                    nc.tensor.matmul(pn[:], hw[:, 2 * H :], h_prev[:],
                                     start=True, stop=True)

                split = rz_bias is not None or CONFIG["split_rz1"]
                if not split:
                    rz = work.tile([H, 2 * BL], DT, tag=f"rz{tag}")
                    nc.scalar.activation(rz[:], prz[:], AF.Sigmoid)
                    r, z = rz[:, 0:BL], rz[:, BL:]
                else:
                    rb = dict(bias=rz_bias[0]) if rz_bias else {}
                    zb = dict(bias=rz_bias[1]) if rz_bias else {}
                    rt = work.tile([H, BL], DT, tag=f"r{tag}")
                    nc.scalar.activation(rt[:], prz[:, 0:BL], AF.Sigmoid, **rb)
                    zt = work.tile([H, BL], DT, tag=f"z{tag}")
                    nc.scalar.activation(zt[:], prz[:, BL:], AF.Sigmoid, **zb)
                    r, z = rt[:], zt[:]

                t2 = work.tile([H, BL], DT, tag=f"t2{tag}")
                if h_prev is not None:
                    # t2 = (hn + b_hh_n) * r
                    nc.vector.scalar_tensor_tensor(
                        t2[:], pn[:], n_hh_bias, r, op0=OP.add, op1=OP.mult
                    )
                else:
                    nc.vector.tensor_scalar_mul(t2[:], r, n_hh_bias)
                n = work.tile([H, BL], DT, tag=f"n{tag}")
                nb = dict(bias=n_ih_bias) if n_ih_bias is not None else {}
                if CONFIG["pre_n_pe"]:
                    # px += I.T @ t2 on the PE, then tanh straight off PSUM
                    nc.tensor.matmul(px[:], ident[:], t2[:], start=False, stop=True)
                    nc.scalar.activation(n[:], px[:], AF.Tanh, **nb)
                else:
                    pre = work.tile([H, BL], F32, tag=f"pre{tag}")
                    nc.vector.tensor_add(pre[:], t2[:], px[:])
                    nc.scalar.activation(n[:], pre[:], AF.Tanh, **nb)
                d = work.tile([H, BL], DT, tag=f"d{tag}")
                if h_prev is not None:
                    ENG[CONFIG["d_eng"]].tensor_sub(d[:], h_prev[:], n[:])
                else:
                    ENG[CONFIG["d_eng"]].tensor_scalar_mul(d[:], n[:], -1.0)
                e = work.tile([H, BL], DT, tag=f"e{tag}")
                ENG[CONFIG["e_eng"]].tensor_mul(e[:], z, d[:])
                h_new = hpool.tile([H, BL], DT, tag=f"h{tag}")
                ENG[CONFIG["h_eng"]].tensor_add(h_new[:], n[:], e[:])
                return h_new

            h1 = None
            h2 = None
            xc = None
            for t in range(n_steps):
                if t % CHUNK == 0:
                    xc = xpool.tile([DA, CHUNK, BL], DT, tag="xc")
                    nc.sync.dma_start(
                        out=xc[:], in_=xT_d[t : t + CHUNK].rearrange("t d b -> d t b")
                    )
                xg = xc[:, t % CHUNK, :]
                h1 = cell("1", h1, xg, l1w, hh1w, None, sbias[:, 0:1], None)
                h2 = cell("2", h2, h1[:], ih2w, hh2w,
                          (sbias[:, 1:2], sbias[:, 2:3]), sbias[:, 3:4],
                          sbias[:, 4:5])

            # ---------------- FC head ------------------------------------
            pfc = pnxpool.tile([NCLS, BL], F32, tag="pn1")
            nc.tensor.matmul(pfc[:], fcw[:], h2[:], start=True, stop=True)
            outs = work.tile([NCLS, BL], F32, tag="outs")
            nc.scalar.activation(outs[:], pfc[:], AF.Identity, bias=fcb[:])
            nc.sync.dma_start(out=out_d[:], in_=outs[:])

    if not for_sim:
        _split_excess_waits(nc)
    return nc


def prep_in_maps(inputs: dict) -> list[dict]:
    """Shard + repack the full-problem numpy inputs into per-core in_maps."""
    x = np.ascontiguousarray(np.asarray(inputs["x"], dtype=np.float32))
    w_ih1 = np.asarray(inputs["w_ih1"], np.float32)
    w_hh1 = np.asarray(inputs["w_hh1"], np.float32)
    b_ih1 = np.asarray(inputs["b_ih1"], np.float32)
    b_hh1 = np.asarray(inputs["b_hh1"], np.float32)
    w_ih2 = np.asarray(inputs["w_ih2"], np.float32)
    w_hh2 = np.asarray(inputs["w_hh2"], np.float32)
    b_ih2 = np.asarray(inputs["b_ih2"], np.float32)
    b_hh2 = np.asarray(inputs["b_hh2"], np.float32)
    fc_w = np.asarray(inputs["fc_w"], np.float32)
    fc_b = np.asarray(inputs["fc_b"], np.float32)

    # x: (4096, 2, 1024) -> per-core time-major transposed [L, 17, BL]
    xr = x.reshape(N_CORES, BL, 2, L, D // 2)  # [core, b, ch, t, j]
    xT = np.empty((N_CORES, L, DA, BL), np.float32)
    xT[:, :, 0 : D // 2, :] = xr[:, :, 0].transpose(0, 2, 3, 1)
    xT[:, :, D // 2 : D, :] = xr[:, :, 1].transpose(0, 2, 3, 1)
    xT[:, :, D, :] = 1.0  # ones row: folds layer-1 biases into the matmul

    # layer-1 combined input-proj weights + bias row.
    # r/z columns carry b_ih1+b_hh1; n columns carry b_ih1 only (b_hh1_n must
    # be applied inside r*(hn+b_hh1_n)).
    l1w = np.empty((DA, G3), np.float32)
    l1w[0:D, :] = w_ih1.T
    bias_row = b_ih1.copy()
    bias_row[0 : 2 * H] += b_hh1[0 : 2 * H]
    l1w[D, :] = bias_row

    bvec = np.stack(
        [
            b_hh1[2 * H : 3 * H],                     # col 0: L1 n-gate hh bias
            (b_ih2 + b_hh2)[0:H],                     # col 1: L2 r bias
            (b_ih2 + b_hh2)[H : 2 * H],               # col 2: L2 z bias
            b_hh2[2 * H : 3 * H],                     # col 3: L2 n-gate hh bias
            b_ih2[2 * H : 3 * H],                     # col 4: L2 n-gate ih bias
        ],
        axis=1,
    ).astype(np.float32)

    if CONFIG["dtype"] == "bf16":
        import ml_dtypes
        ndt = np.dtype(ml_dtypes.bfloat16)
    else:
        ndt = _NP_DT[CONFIG["dtype"]]
    shared = {
        "l1w": np.ascontiguousarray(l1w).astype(ndt),
        "hh1w": np.ascontiguousarray(w_hh1.T).astype(ndt),
        "ih2w": np.ascontiguousarray(w_ih2.T).astype(ndt),
        "hh2w": np.ascontiguousarray(w_hh2.T).astype(ndt),
        "bvec": bvec,
        "fcw": np.ascontiguousarray(fc_w.T).astype(ndt),
        "fcb": np.ascontiguousarray(fc_b[:, None]),
        "ident": np.eye(H, dtype=np.float32).astype(ndt),
    }
    return [{"xT": np.ascontiguousarray(xT[c]).astype(ndt), **shared}
            for c in range(N_CORES)]


def assemble_output(results: list[dict]) -> np.ndarray:
    # per-core outT [24, BL] -> (4096, 24)
    return np.concatenate([r["outT"].T for r in results], axis=0).astype(np.float32)


_NC_CACHE = None


def kernel(**inputs) -> np.ndarray:
    global _NC_CACHE
    if _NC_CACHE is None:
        _NC_CACHE = build_program()
    in_maps = prep_in_maps(inputs)
    res = run_bass_kernel_spmd(_NC_CACHE, in_maps, list(range(N_CORES)))
    return assemble_output(res.results)



# revision 2
# speedup vs baseline: 1.8989x; 1.8989x over previous
"""Two-layer GRU (16->128->128) + FC(128->24) head on 8 Trainium2 NeuronCores.

Strategy: data-parallel over the batch (4096 -> 512 per core); tiny weights
replicated. On each core the hidden state lives transposed in SBUF as
[H=128 partitions, B=512 free]. Per time step, gate pre-activations are
accumulated in PSUM by fp16 matmuls (input-projection + recurrent + biases
folded in), sigmoid/tanh run on the scalar engine with per-partition bias
APs, and the cell update is spread across vector + gpsimd engines.

All per-core inputs (time-major packed x, weights, biases, identity) are
packed into ONE [NROWS, 512] f16 DRAM tensor per core: the PJRT dispatch
path pays a large fixed cost per argument buffer, so a single blob argument
is much cheaper to launch than nine separate tensors. f32 regions (biases)
are stored bit-exact inside the f16 blob and read back via bitcast views.

Self-contained: hardcodes all shapes; host-side prep only reshapes/transposes
numpy arrays (sharding + time-major packing of x, weight transposes).
"""

import numpy as np

import bass_rust
import concourse.bass as bass
import concourse.mybir as mybir
from concourse.tile import TileContext
from concourse.vector_clock import ScopedClock
from concourse.bass_utils import run_bass_kernel_spmd

N_CORES = 8
B_TOT = 4096
L = 128          # sequence length (= 2*1024/16)
D = 16           # per-step input features
DA = 17          # + ones row (bias folding for layer 1)
H = 128          # hidden
G3 = 3 * H       # 384 stacked gates (r, z, n)
BL = B_TOT // N_CORES  # 512 batch per core
NCLS = 24
CHUNK = 8        # time steps of x staged into SBUF per DMA

F32 = mybir.dt.float32
F32R = mybir.dt.float32r
F16 = mybir.dt.float16
BF16 = mybir.dt.bfloat16
AF = mybir.ActivationFunctionType
OP = mybir.AluOpType

# Blob row layout ([NROWS, 512] f16 per core; f32 regions via bitcast->
# [NROWS, 256] f32 view, bit-exact).
ROW_XT = 0                     # 128*17 rows, [t,d] t-major, full 512 cols
ROW_L1W = ROW_XT + L * DA      # 17 rows, cols 0:384
ROW_HH1 = ROW_L1W + DA         # 128 rows, cols 0:384
ROW_IH2 = ROW_HH1 + H          # 128 rows, cols 0:384
ROW_HH2 = ROW_IH2 + H          # 128 rows, cols 0:384
ROW_FCW = ROW_HH2 + H          # 128 rows, cols 0:24
ROW_IDENT = ROW_FCW + H        # 128 rows, cols 0:128
ROW_BVEC = ROW_IDENT + H       # 128 rows, f32 cols 0:5
ROW_FCB = ROW_BVEC + H         # 24 rows, f32 col 0
NROWS = ROW_FCB + NCLS + 1     # 2986

# Tunables (grid-searched via TimelineSim, validated on HW).
CONFIG = {
    "pre_n_pe": True,    # accumulate t2 into P_x via PE identity matmul
    "split_rz1": True,  # separate r/z sigmoids for layer 1
    "d_eng": "v",        # engine for d = h - n
    "e_eng": "v",        # engine for e = z * d
    "h_eng": "v",        # engine for h' = n + e
}

DT = F16


class SplitDrainTileContext(TileContext):
    """Walrus (CoreV3) rejects instructions carrying >2 sync waits; Tile's
    kernel-tail drain accumulates one wait per outstanding engine/DMA-queue
    sem. Split them across a chain of drains (1 wait each)."""

    def _drain_and_barrier(self, tick_clock, wait_clock):
        nc = self.nc
        drain_inst = nc.sync.drain()
        wait_clock.add_sem_waits(
            drain_inst.ins, ScopedClock({None: tick_clock.global_clock})
        )
        si = drain_inst.ins.sync_info
        if si is not None and len(si.on_wait) > 1:
            waits = list(si.on_wait)
            si.on_wait = waits[:1]
            for w in waits[1:]:
                d2 = nc.sync.drain()
                d2.ins.sync_info = bass_rust.SyncInfo(on_wait=[w], on_update=[])
        nc.all_engine_barrier()
        popped = nc._tile_sem_poison_stack.pop()
        assert popped is self._sem_poison
        nc.clear_and_free_semaphores(list(self.sems.allocated().values()))
        nc.all_engine_barrier()


def _split_excess_waits(nc: bass.Bass, max_waits: int = 1) -> None:
    """Walrus (CoreV3 setupSyncWait) accepts at most 2 sem waits per
    instruction; Tile occasionally attaches 3+. Hoist the excess onto
    EventSemaphore instructions inserted right before the offender on the
    same engine (serial waits AND together)."""
    n = 0
    for fn in nc.m.functions:
        for bb in fn.blocks:
            out = []
            dirty = False
            for inst in bb.instructions:
                si = inst.sync_info
                if si is not None and len(si.on_wait) > max_waits:
                    waits = list(si.on_wait)
                    extra = waits[: len(waits) - max_waits]
                    for w in extra:
                        ev = mybir.InstEventSemaphore(
                            name=f"evs-waitsplit-{n}", ins=[], outs=[]
                        )
                        n += 1
                        ev.engine = inst.engine
                        ev.sync_info = bass_rust.SyncInfo(
                            on_wait=[w], on_update=[]
                        )
                        out.append(ev)
                    si.on_wait = waits[len(waits) - max_waits :]
                    dirty = True
                out.append(inst)
            if dirty:
                bb.instructions = out


def build_program(for_sim: bool = False, n_steps: int = L) -> bass.Bass:
    # for_sim: skip the walrus wait-limit workarounds (post-hoc IR mutations
    # that CoreSim's bookkeeping doesn't understand); semantics identical.
    nc = bass.Bass()

    blob_d = nc.declare_dram_parameter("blob", [NROWS, 512], DT, isOutput=False)
    b32 = blob_d.bitcast(F32)  # [NROWS, 256] f32 view of the same bytes
    out_d = nc.declare_dram_parameter("outT", [NCLS, BL], F32, isOutput=True)

    tc_cls = TileContext if for_sim else SplitDrainTileContext
    with tc_cls(nc) as tc:
        with (
            tc.tile_pool(name="singles", bufs=1) as singles,
            tc.tile_pool(name="xchunks", bufs=3) as xpool,
            tc.tile_pool(name="hstate", bufs=2) as hpool,
            tc.tile_pool(name="work", bufs=3) as work,
            tc.tile_pool(name="prz", bufs=1, space="PSUM") as przpool,
            tc.tile_pool(name="pnx", bufs=1, space="PSUM") as pnxpool,
        ):
            # --- constant loads -------------------------------------------
            l1w = singles.tile([DA, G3], DT, tag="l1w")
            hh1w = singles.tile([H, G3], DT, tag="hh1w")
            ih2w = singles.tile([H, G3], DT, tag="ih2w")
            hh2w = singles.tile([H, G3], DT, tag="hh2w")
            sbias = singles.tile([H, 5], F32, tag="sbias")
            fcw = singles.tile([H, NCLS], DT, tag="fcw")
            fcb = singles.tile([NCLS, 1], F32, tag="fcb")
            ident = singles.tile([H, H], DT, tag="ident")
            nc.sync.dma_start(out=ident[:], in_=blob_d[ROW_IDENT : ROW_IDENT + H, 0:H])
            nc.sync.dma_start(out=l1w[:], in_=blob_d[ROW_L1W : ROW_L1W + DA, 0:G3])
            nc.sync.dma_start(out=hh1w[:], in_=blob_d[ROW_HH1 : ROW_HH1 + H, 0:G3])
            nc.sync.dma_start(out=ih2w[:], in_=blob_d[ROW_IH2 : ROW_IH2 + H, 0:G3])
            nc.sync.dma_start(out=hh2w[:], in_=blob_d[ROW_HH2 : ROW_HH2 + H, 0:G3])
            nc.sync.dma_start(out=sbias[:], in_=b32[ROW_BVEC : ROW_BVEC + H, 0:5])
            nc.sync.dma_start(out=fcw[:], in_=blob_d[ROW_FCW : ROW_FCW + H, 0:NCLS])
            nc.sync.dma_start(out=fcb[:], in_=b32[ROW_FCB : ROW_FCB + NCLS, 0:1])

            ENG = {"v": nc.vector, "g": nc.gpsimd}

            def cell(tag, h_prev, x_rhs, xw, hw, rz_bias, n_hh_bias, n_ih_bias):
                """One GRU cell step, transposed layout [H partitions, BL free].

                h_prev: [H, BL] DT tile or None (t=0 => h=0, recurrent
                matmuls skipped). x_rhs: [K, BL] DT rhs for the input
                projection with lhsT xw [K, G3]; hw: [H, G3] recurrent lhsT.
                rz_bias: None (folded into xw) or (r_bias_ap, z_bias_ap).
                Returns the new [H, BL] DT hidden tile.
                """
                prz = przpool.tile([H, 2 * BL], F32, tag=f"prz{tag}")
                pn = pnxpool.tile([H, BL], F32, tag=f"pn{tag}")
                px = pnxpool.tile([H, BL], F32, tag=f"px{tag}")
                nc.tensor.matmul(prz[:, 0:BL], xw[:, 0:H], x_rhs,
                                 start=True, stop=h_prev is None)
                nc.tensor.matmul(prz[:, BL:], xw[:, H : 2 * H], x_rhs,
                                 start=True, stop=h_prev is None)
                if h_prev is not None:
                    nc.tensor.matmul(prz[:, 0:BL], hw[:, 0:H], h_prev[:],
                                     start=False, stop=True)
                    nc.tensor.matmul(prz[:, BL:], hw[:, H : 2 * H], h_prev[:],
                                     start=False, stop=True)
                nc.tensor.matmul(px[:], xw[:, 2 * H :], x_rhs, start=True,
                                 stop=not CONFIG["pre_n_pe"])
                if h_prev is not None:
                    nc.tensor.matmul(pn[:], hw[:, 2 * H :], h_prev[:],
                                     start=True, stop=True)

                split = rz_bias is not None or CONFIG["split_rz1"]
                if not split:
                    rz = work.tile([H, 2 * BL], DT, tag=f"rz{tag}")
                    nc.scalar.activation(rz[:], prz[:], AF.Sigmoid)
                    r, z = rz[:, 0:BL], rz[:, BL:]
                else:
                    rb = dict(bias=rz_bias[0]) if rz_bias else {}
                    zb = dict(bias=rz_bias[1]) if rz_bias else {}
                    rt = work.tile([H, BL], DT, tag=f"r{tag}")
                    nc.scalar.activation(rt[:], prz[:, 0:BL], AF.Sigmoid, **rb)
                    zt = work.tile([H, BL], DT, tag=f"z{tag}")
                    nc.scalar.activation(zt[:], prz[:, BL:], AF.Sigmoid, **zb)
                    r, z = rt[:], zt[:]

                t2 = work.tile([H, BL], DT, tag=f"t2{tag}")
                if h_prev is not None:
                    # t2 = (hn + b_hh_n) * r
                    nc.vector.scalar_tensor_tensor(
                        t2[:], pn[:], n_hh_bias, r, op0=OP.add, op1=OP.mult
                    )
                else:
                    nc.vector.tensor_scalar_mul(t2[:], r, n_hh_bias)
                n = work.tile([H, BL], DT, tag=f"n{tag}")
                nb = dict(bias=n_ih_bias) if n_ih_bias is not None else {}
                if CONFIG["pre_n_pe"]:
                    # px += I.T @ t2 on the PE, then tanh straight off PSUM
                    nc.tensor.matmul(px[:], ident[:], t2[:], start=False, stop=True)
                    nc.scalar.activation(n[:], px[:], AF.Tanh, **nb)
                else:
                    pre = work.tile([H, BL], F32, tag=f"pre{tag}")
                    nc.vector.tensor_add(pre[:], t2[:], px[:])
                    nc.scalar.activation(n[:], pre[:], AF.Tanh, **nb)
                d = work.tile([H, BL], DT, tag=f"d{tag}")
                if h_prev is not None:
                    ENG[CONFIG["d_eng"]].tensor_sub(d[:], h_prev[:], n[:])
                else:
                    ENG[CONFIG["d_eng"]].tensor_scalar_mul(d[:], n[:], -1.0)
                e = work.tile([H, BL], DT, tag=f"e{tag}")
                ENG[CONFIG["e_eng"]].tensor_mul(e[:], z, d[:])
                h_new = hpool.tile([H, BL], DT, tag=f"h{tag}")
                ENG[CONFIG["h_eng"]].tensor_add(h_new[:], n[:], e[:])
                return h_new

            h1 = None
            h2 = None
            xc = None
            for t in range(n_steps):
                if t % CHUNK == 0:
                    xc = xpool.tile([DA, CHUNK, BL], DT, tag="xc")
                    nc.sync.dma_start(
                        out=xc[:],
                        in_=blob_d[t * DA : (t + CHUNK) * DA].rearrange(
                            "(t d) b -> d t b", t=CHUNK
                        ),
                    )
                xg = xc[:, t % CHUNK, :]
                h1 = cell("1", h1, xg, l1w, hh1w, None, sbias[:, 0:1], None)
                h2 = cell("2", h2, h1[:], ih2w, hh2w,
                          (sbias[:, 1:2], sbias[:, 2:3]), sbias[:, 3:4],
                          sbias[:, 4:5])

            # ---------------- FC head ------------------------------------
            pfc = pnxpool.tile([NCLS, BL], F32, tag="pn1")
            nc.tensor.matmul(pfc[:], fcw[:], h2[:], start=True, stop=True)
            outs = work.tile([NCLS, BL], F32, tag="outs")
            nc.scalar.activation(outs[:], pfc[:], AF.Identity, bias=fcb[:])
            nc.sync.dma_start(out=out_d[:], in_=outs[:])

    if not for_sim:
        _split_excess_waits(nc)
    return nc


def prep_in_maps(inputs: dict) -> list[dict]:
    """Shard + repack the full-problem numpy inputs into per-core blobs."""
    x = np.ascontiguousarray(np.asarray(inputs["x"], dtype=np.float32))
    w_ih1 = np.asarray(inputs["w_ih1"], np.float32)
    w_hh1 = np.asarray(inputs["w_hh1"], np.float32)
    b_ih1 = np.asarray(inputs["b_ih1"], np.float32)
    b_hh1 = np.asarray(inputs["b_hh1"], np.float32)
    w_ih2 = np.asarray(inputs["w_ih2"], np.float32)
    w_hh2 = np.asarray(inputs["w_hh2"], np.float32)
    b_ih2 = np.asarray(inputs["b_ih2"], np.float32)
    b_hh2 = np.asarray(inputs["b_hh2"], np.float32)
    fc_w = np.asarray(inputs["fc_w"], np.float32)
    fc_b = np.asarray(inputs["fc_b"], np.float32)

    blob = np.zeros((N_CORES, NROWS, 512), np.float16)

    # x: (4096, 2, 1024) -> per-core time-major transposed rows [t,d] -> b
    xr = x.reshape(N_CORES, BL, 2, L, D // 2)  # [core, b, ch, t, j]
    bxT = blob[:, ROW_XT : ROW_XT + L * DA, :].reshape(N_CORES, L, DA, BL)
    bxT[:, :, 0 : D // 2, :] = xr[:, :, 0].transpose(0, 2, 3, 1)
    bxT[:, :, D // 2 : D, :] = xr[:, :, 1].transpose(0, 2, 3, 1)
    bxT[:, :, D, :] = 1.0  # ones row: folds layer-1 biases into the matmul

    # layer-1 combined input-proj weights + bias row.
    # r/z columns carry b_ih1+b_hh1; n columns carry b_ih1 only (b_hh1_n must
    # be applied inside r*(hn+b_hh1_n)).
    l1w = np.empty((DA, G3), np.float32)
    l1w[0:D, :] = w_ih1.T
    bias_row = b_ih1.copy()
    bias_row[0 : 2 * H] += b_hh1[0 : 2 * H]
    l1w[D, :] = bias_row

    bvec = np.stack(
        [
            b_hh1[2 * H : 3 * H],                     # col 0: L1 n-gate hh bias
            (b_ih2 + b_hh2)[0:H],                     # col 1: L2 r bias
            (b_ih2 + b_hh2)[H : 2 * H],               # col 2: L2 z bias
            b_hh2[2 * H : 3 * H],                     # col 3: L2 n-gate hh bias
            b_ih2[2 * H : 3 * H],                     # col 4: L2 n-gate ih bias
        ],
        axis=1,
    ).astype(np.float32)

    blob[:, ROW_L1W : ROW_L1W + DA, 0:G3] = l1w.astype(np.float16)
    blob[:, ROW_HH1 : ROW_HH1 + H, 0:G3] = w_hh1.T.astype(np.float16)
    blob[:, ROW_IH2 : ROW_IH2 + H, 0:G3] = w_ih2.T.astype(np.float16)
    blob[:, ROW_HH2 : ROW_HH2 + H, 0:G3] = w_hh2.T.astype(np.float16)
    blob[:, ROW_FCW : ROW_FCW + H, 0:NCLS] = fc_w.T.astype(np.float16)
    blob[:, ROW_IDENT : ROW_IDENT + H, 0:H] = np.eye(H, dtype=np.float16)
    # f32 regions, stored bit-exact as pairs of f16 slots
    blob[:, ROW_BVEC : ROW_BVEC + H, 0:10] = bvec.view(np.float16)
    blob[:, ROW_FCB : ROW_FCB + NCLS, 0:2] = fc_b[:, None].view(np.float16)

    return [{"blob": blob[c]} for c in range(N_CORES)]


def assemble_output(results: list[dict]) -> np.ndarray:
    # per-core outT [24, BL] -> (4096, 24)
    return np.concatenate([r["outT"].T for r in results], axis=0).astype(np.float32)


_NC_CACHE = None


def kernel(**inputs) -> np.ndarray:
    global _NC_CACHE
    if _NC_CACHE is None:
        _NC_CACHE = build_program()
    in_maps = prep_in_maps(inputs)
    res = run_bass_kernel_spmd(_NC_CACHE, in_maps, list(range(N_CORES)))
    return assemble_output(res.results)


# revision 4
# speedup vs baseline: 2.0639x; 1.0869x over previous
"""Two-layer GRU (16->128->128) + FC(128->24) head on 8 Trainium2 NeuronCores.

Strategy: data-parallel over the batch (4096 -> 512 per core); tiny weights
replicated. On each core the hidden state lives transposed in SBUF as
[H=128 partitions, B=512 free]. Per time step, gate pre-activations are
accumulated in PSUM by fp16 matmuls (input-projection + recurrent + biases
folded in), sigmoid/tanh run on the scalar engine with per-partition bias
APs, and the cell update is spread across vector + gpsimd engines.

All per-core inputs (time-major packed x, weights, biases, identity) are
packed into ONE [NROWS, 512] f16 DRAM tensor per core: the PJRT dispatch
path pays a large fixed cost per argument buffer, so a single blob argument
is much cheaper to launch than nine separate tensors. f32 regions (biases)
are stored bit-exact inside the f16 blob and read back via bitcast views.

Self-contained: hardcodes all shapes; host-side prep only reshapes/transposes
numpy arrays (sharding + time-major packing of x, weight transposes).
"""

import numpy as np

import bass_rust
import concourse.bass as bass
import concourse.mybir as mybir
from concourse.tile import TileContext
from concourse.vector_clock import ScopedClock
from concourse.bass_utils import run_bass_kernel_spmd

N_CORES = 8
B_TOT = 4096
L = 128          # sequence length (= 2*1024/16)
D = 16           # per-step input features
DA = 17          # + ones row (bias folding for layer 1)
H = 128          # hidden
G3 = 3 * H       # 384 stacked gates (r, z, n)
BL = B_TOT // N_CORES  # 512 batch per core
NCLS = 24
CHUNK = 8        # time steps of x staged into SBUF per DMA

F32 = mybir.dt.float32
F32R = mybir.dt.float32r
F16 = mybir.dt.float16
BF16 = mybir.dt.bfloat16
AF = mybir.ActivationFunctionType
OP = mybir.AluOpType

# Blob row layout ([NROWS, 512] f16 per core; f32 regions via bitcast->
# [NROWS, 256] f32 view, bit-exact).
ROW_XT = 0                     # 128*17 rows, [t,d] t-major, full 512 cols
ROW_L1W = ROW_XT + L * DA      # 17 rows, cols 0:384
ROW_HH1 = ROW_L1W + DA         # 128 rows, cols 0:384
ROW_IH2 = ROW_HH1 + H          # 128 rows, cols 0:384
ROW_HH2 = ROW_IH2 + H          # 128 rows, cols 0:384
ROW_FCW = ROW_HH2 + H          # 128 rows, cols 0:24
ROW_IDENT = ROW_FCW + H        # 128 rows, cols 0:128
ROW_BVEC = ROW_IDENT + H       # 128 rows, f32 cols 0:5
ROW_FCB = ROW_BVEC + H         # 24 rows, f32 col 0
NROWS = ROW_FCB + NCLS + 1     # 2986

# Tunables (grid-searched via TimelineSim, validated on HW).
CONFIG = {
    "pre_n_pe": True,    # accumulate t2 into P_x via PE identity matmul
    "split_rz1": True,  # separate r/z sigmoids for layer 1
    "d_eng": "v",        # engine for d = h - n
    "e_eng": "v",        # engine for e = z * d
    "h_eng": "v",        # engine for h' = n + e
}

DT = F16


class SplitDrainTileContext(TileContext):
    """Walrus (CoreV3) rejects instructions carrying >2 sync waits; Tile's
    kernel-tail drain accumulates one wait per outstanding engine/DMA-queue
    sem. Split them across a chain of drains (1 wait each)."""

    def _drain_and_barrier(self, tick_clock, wait_clock):
        nc = self.nc
        drain_inst = nc.sync.drain()
        wait_clock.add_sem_waits(
            drain_inst.ins, ScopedClock({None: tick_clock.global_clock})
        )
        si = drain_inst.ins.sync_info
        if si is not None and len(si.on_wait) > 1:
            waits = list(si.on_wait)
            si.on_wait = waits[:1]
            for w in waits[1:]:
                d2 = nc.sync.drain()
                d2.ins.sync_info = bass_rust.SyncInfo(on_wait=[w], on_update=[])
        nc.all_engine_barrier()
        popped = nc._tile_sem_poison_stack.pop()
        assert popped is self._sem_poison
        nc.clear_and_free_semaphores(list(self.sems.allocated().values()))
        nc.all_engine_barrier()


def _split_excess_waits(nc: bass.Bass, max_waits: int = 1) -> None:
    """Walrus (CoreV3 setupSyncWait) accepts at most 2 sem waits per
    instruction; Tile occasionally attaches 3+. Hoist the excess onto
    EventSemaphore instructions inserted right before the offender on the
    same engine (serial waits AND together)."""
    n = 0
    for fn in nc.m.functions:
        for bb in fn.blocks:
            out = []
            dirty = False
            for inst in bb.instructions:
                si = inst.sync_info
                if si is not None and len(si.on_wait) > max_waits:
                    waits = list(si.on_wait)
                    extra = waits[: len(waits) - max_waits]
                    for w in extra:
                        ev = mybir.InstEventSemaphore(
                            name=f"evs-waitsplit-{n}", ins=[], outs=[]
                        )
                        n += 1
                        ev.engine = inst.engine
                        ev.sync_info = bass_rust.SyncInfo(
                            on_wait=[w], on_update=[]
                        )
                        out.append(ev)
                    si.on_wait = waits[len(waits) - max_waits :]
                    dirty = True
                out.append(inst)
            if dirty:
                bb.instructions = out


def build_program(for_sim: bool = False, n_steps: int = L) -> bass.Bass:
    # for_sim: skip the walrus wait-limit workarounds (post-hoc IR mutations
    # that CoreSim's bookkeeping doesn't understand); semantics identical.
    nc = bass.Bass()

    blob_d = nc.declare_dram_parameter("blob", [NROWS, 512], DT, isOutput=False)
    b32 = blob_d.bitcast(F32)  # [NROWS, 256] f32 view of the same bytes
    # f16 output: the per-call donated-zero upload for the output buffer is
    # paid per dispatch at ~50 MB/s, so halving output bytes saves ~4 ms.
    out_d = nc.declare_dram_parameter("outT", [NCLS, BL], F16, isOutput=True)

    tc_cls = TileContext if for_sim else SplitDrainTileContext
    with tc_cls(nc) as tc:
        with (
            tc.tile_pool(name="singles", bufs=1) as singles,
            tc.tile_pool(name="xchunks", bufs=3) as xpool,
            tc.tile_pool(name="hstate", bufs=2) as hpool,
            tc.tile_pool(name="work", bufs=3) as work,
            tc.tile_pool(name="prz", bufs=1, space="PSUM") as przpool,
            tc.tile_pool(name="pnx", bufs=1, space="PSUM") as pnxpool,
        ):
            # --- constant loads -------------------------------------------
            l1w = singles.tile([DA, G3], DT, tag="l1w")
            hh1w = singles.tile([H, G3], DT, tag="hh1w")
            ih2w = singles.tile([H, G3], DT, tag="ih2w")
            hh2w = singles.tile([H, G3], DT, tag="hh2w")
            sbias = singles.tile([H, 5], F32, tag="sbias")
            fcw = singles.tile([H, NCLS], DT, tag="fcw")
            fcb = singles.tile([NCLS, 1], F32, tag="fcb")
            ident = singles.tile([H, H], DT, tag="ident")
            nc.sync.dma_start(out=ident[:], in_=blob_d[ROW_IDENT : ROW_IDENT + H, 0:H])
            nc.sync.dma_start(out=l1w[:], in_=blob_d[ROW_L1W : ROW_L1W + DA, 0:G3])
            nc.sync.dma_start(out=hh1w[:], in_=blob_d[ROW_HH1 : ROW_HH1 + H, 0:G3])
            nc.sync.dma_start(out=ih2w[:], in_=blob_d[ROW_IH2 : ROW_IH2 + H, 0:G3])
            nc.sync.dma_start(out=hh2w[:], in_=blob_d[ROW_HH2 : ROW_HH2 + H, 0:G3])
            nc.sync.dma_start(out=sbias[:], in_=b32[ROW_BVEC : ROW_BVEC + H, 0:5])
            nc.sync.dma_start(out=fcw[:], in_=blob_d[ROW_FCW : ROW_FCW + H, 0:NCLS])
            nc.sync.dma_start(out=fcb[:], in_=b32[ROW_FCB : ROW_FCB + NCLS, 0:1])

            ENG = {"v": nc.vector, "g": nc.gpsimd}

            def cell(tag, h_prev, x_rhs, xw, hw, rz_bias, n_hh_bias, n_ih_bias):
                """One GRU cell step, transposed layout [H partitions, BL free].

                h_prev: [H, BL] DT tile or None (t=0 => h=0, recurrent
                matmuls skipped). x_rhs: [K, BL] DT rhs for the input
                projection with lhsT xw [K, G3]; hw: [H, G3] recurrent lhsT.
                rz_bias: None (folded into xw) or (r_bias_ap, z_bias_ap).
                Returns the new [H, BL] DT hidden tile.
                """
                prz = przpool.tile([H, 2 * BL], F32, tag=f"prz{tag}")
                pn = pnxpool.tile([H, BL], F32, tag=f"pn{tag}")
                px = pnxpool.tile([H, BL], F32, tag=f"px{tag}")
                nc.tensor.matmul(prz[:, 0:BL], xw[:, 0:H], x_rhs,
                                 start=True, stop=h_prev is None)
                nc.tensor.matmul(prz[:, BL:], xw[:, H : 2 * H], x_rhs,
                                 start=True, stop=h_prev is None)
                if h_prev is not None:
                    nc.tensor.matmul(prz[:, 0:BL], hw[:, 0:H], h_prev[:],
                                     start=False, stop=True)
                    nc.tensor.matmul(prz[:, BL:], hw[:, H : 2 * H], h_prev[:],
                                     start=False, stop=True)
                nc.tensor.matmul(px[:], xw[:, 2 * H :], x_rhs, start=True,
                                 stop=not CONFIG["pre_n_pe"])
                if h_prev is not None:
                    nc.tensor.matmul(pn[:], hw[:, 2 * H :], h_prev[:],
                                     start=True, stop=True)

                split = rz_bias is not None or CONFIG["split_rz1"]
                if not split:
                    rz = work.tile([H, 2 * BL], DT, tag=f"rz{tag}")
                    nc.scalar.activation(rz[:], prz[:], AF.Sigmoid)
                    r, z = rz[:, 0:BL], rz[:, BL:]
                else:
                    rb = dict(bias=rz_bias[0]) if rz_bias else {}
                    zb = dict(bias=rz_bias[1]) if rz_bias else {}
                    rt = work.tile([H, BL], DT, tag=f"r{tag}")
                    nc.scalar.activation(rt[:], prz[:, 0:BL], AF.Sigmoid, **rb)
                    zt = work.tile([H, BL], DT, tag=f"z{tag}")
                    nc.scalar.activation(zt[:], prz[:, BL:], AF.Sigmoid, **zb)
                    r, z = rt[:], zt[:]

                t2 = work.tile([H, BL], DT, tag=f"t2{tag}")
                if h_prev is not None:
                    # t2 = (hn + b_hh_n) * r
                    nc.vector.scalar_tensor_tensor(
                        t2[:], pn[:], n_hh_bias, r, op0=OP.add, op1=OP.mult
                    )
                else:
                    nc.vector.tensor_scalar_mul(t2[:], r, n_hh_bias)
                n = work.tile([H, BL], DT, tag=f"n{tag}")
                nb = dict(bias=n_ih_bias) if n_ih_bias is not None else {}
                if CONFIG["pre_n_pe"]:
                    # px += I.T @ t2 on the PE, then tanh straight off PSUM
                    nc.tensor.matmul(px[:], ident[:], t2[:], start=False, stop=True)
                    nc.scalar.activation(n[:], px[:], AF.Tanh, **nb)
                else:
                    pre = work.tile([H, BL], F32, tag=f"pre{tag}")
                    nc.vector.tensor_add(pre[:], t2[:], px[:])
                    nc.scalar.activation(n[:], pre[:], AF.Tanh, **nb)
                d = work.tile([H, BL], DT, tag=f"d{tag}")
                if h_prev is not None:
                    ENG[CONFIG["d_eng"]].tensor_sub(d[:], h_prev[:], n[:])
                else:
                    ENG[CONFIG["d_eng"]].tensor_scalar_mul(d[:], n[:], -1.0)
                e = work.tile([H, BL], DT, tag=f"e{tag}")
                ENG[CONFIG["e_eng"]].tensor_mul(e[:], z, d[:])
                h_new = hpool.tile([H, BL], DT, tag=f"h{tag}")
                ENG[CONFIG["h_eng"]].tensor_add(h_new[:], n[:], e[:])
                return h_new

            h1 = None
            h2 = None
            xc = None
            for t in range(n_steps):
                if t % CHUNK == 0:
                    xc = xpool.tile([DA, CHUNK, BL], DT, tag="xc")
                    nc.sync.dma_start(
                        out=xc[:],
                        in_=blob_d[t * DA : (t + CHUNK) * DA].rearrange(
                            "(t d) b -> d t b", t=CHUNK
                        ),
                    )
                xg = xc[:, t % CHUNK, :]
                h1 = cell("1", h1, xg, l1w, hh1w, None, sbias[:, 0:1], None)
                h2 = cell("2", h2, h1[:], ih2w, hh2w,
                          (sbias[:, 1:2], sbias[:, 2:3]), sbias[:, 3:4],
                          sbias[:, 4:5])

            # ---------------- FC head ------------------------------------
            pfc = pnxpool.tile([NCLS, BL], F32, tag="pn1")
            nc.tensor.matmul(pfc[:], fcw[:], h2[:], start=True, stop=True)
            outs = work.tile([NCLS, BL], F16, tag="outs")
            nc.scalar.activation(outs[:], pfc[:], AF.Identity, bias=fcb[:])
            nc.sync.dma_start(out=out_d[:], in_=outs[:])

    if not for_sim:
        _split_excess_waits(nc)
    return nc


def prep_in_maps(inputs: dict) -> list[dict]:
    """Shard + repack the full-problem numpy inputs into per-core blobs."""
    x = np.ascontiguousarray(np.asarray(inputs["x"], dtype=np.float32))
    w_ih1 = np.asarray(inputs["w_ih1"], np.float32)
    w_hh1 = np.asarray(inputs["w_hh1"], np.float32)
    b_ih1 = np.asarray(inputs["b_ih1"], np.float32)
    b_hh1 = np.asarray(inputs["b_hh1"], np.float32)
    w_ih2 = np.asarray(inputs["w_ih2"], np.float32)
    w_hh2 = np.asarray(inputs["w_hh2"], np.float32)
    b_ih2 = np.asarray(inputs["b_ih2"], np.float32)
    b_hh2 = np.asarray(inputs["b_hh2"], np.float32)
    fc_w = np.asarray(inputs["fc_w"], np.float32)
    fc_b = np.asarray(inputs["fc_b"], np.float32)

    blob = np.zeros((N_CORES, NROWS, 512), np.float16)

    # x: (4096, 2, 1024) -> per-core time-major transposed rows [t,d] -> b
    xr = x.reshape(N_CORES, BL, 2, L, D // 2)  # [core, b, ch, t, j]
    bxT = blob[:, ROW_XT : ROW_XT + L * DA, :].reshape(N_CORES, L, DA, BL)
    bxT[:, :, 0 : D // 2, :] = xr[:, :, 0].transpose(0, 2, 3, 1)
    bxT[:, :, D // 2 : D, :] = xr[:, :, 1].transpose(0, 2, 3, 1)
    bxT[:, :, D, :] = 1.0  # ones row: folds layer-1 biases into the matmul

    # layer-1 combined input-proj weights + bias row.
    # r/z columns carry b_ih1+b_hh1; n columns carry b_ih1 only (b_hh1_n must
    # be applied inside r*(hn+b_hh1_n)).
    l1w = np.empty((DA, G3), np.float32)
    l1w[0:D, :] = w_ih1.T
    bias_row = b_ih1.copy()
    bias_row[0 : 2 * H] += b_hh1[0 : 2 * H]
    l1w[D, :] = bias_row

    bvec = np.stack(
        [
            b_hh1[2 * H : 3 * H],                     # col 0: L1 n-gate hh bias
            (b_ih2 + b_hh2)[0:H],                     # col 1: L2 r bias
            (b_ih2 + b_hh2)[H : 2 * H],               # col 2: L2 z bias
            b_hh2[2 * H : 3 * H],                     # col 3: L2 n-gate hh bias
            b_ih2[2 * H : 3 * H],                     # col 4: L2 n-gate ih bias
        ],
        axis=1,
    ).astype(np.float32)

    blob[:, ROW_L1W : ROW_L1W + DA, 0:G3] = l1w.astype(np.float16)
    blob[:, ROW_HH1 : ROW_HH1 + H, 0:G3] = w_hh1.T.astype(np.float16)
    blob[:, ROW_IH2 : ROW_IH2 + H, 0:G3] = w_ih2.T.astype(np.float16)
    blob[:, ROW_HH2 : ROW_HH2 + H, 0:G3] = w_hh2.T.astype(np.float16)
    blob[:, ROW_FCW : ROW_FCW + H, 0:NCLS] = fc_w.T.astype(np.float16)
    blob[:, ROW_IDENT : ROW_IDENT + H, 0:H] = np.eye(H, dtype=np.float16)
    # f32 regions, stored bit-exact as pairs of f16 slots
    blob[:, ROW_BVEC : ROW_BVEC + H, 0:10] = bvec.view(np.float16)
    blob[:, ROW_FCB : ROW_FCB + NCLS, 0:2] = fc_b[:, None].view(np.float16)

    return [{"blob": blob[c]} for c in range(N_CORES)]


def assemble_output(results: list[dict]) -> np.ndarray:
    # per-core f16 outT [24, BL] -> (4096, 24) f32
    return np.concatenate([r["outT"].T for r in results], axis=0).astype(np.float32)


_NC_CACHE = None


def kernel(**inputs) -> np.ndarray:
    global _NC_CACHE
    if _NC_CACHE is None:
        _NC_CACHE = build_program()
    in_maps = prep_in_maps(inputs)
    res = run_bass_kernel_spmd(_NC_CACHE, in_maps, list(range(N_CORES)))
    return assemble_output(res.results)


# revision 8
# speedup vs baseline: 2.6652x; 1.2913x over previous
"""Two-layer GRU (16->128->128) + FC(128->24) head on 8 Trainium2 NeuronCores.

Strategy: data-parallel over the batch (4096 -> 512 per core); tiny weights
replicated. On each core the hidden state lives transposed in SBUF as
[H=128 partitions, B=512 free]. Per time step, gate pre-activations are
accumulated in PSUM by fp16 matmuls (input-projection + recurrent + biases
folded in), sigmoid/tanh run on the scalar engine with per-partition bias
APs, and the cell update is spread across vector + gpsimd engines.

All per-core inputs (time-major packed x, weights, biases, identity) are
packed into ONE [NROWS, 512] f16 DRAM tensor per core: the PJRT dispatch
path pays a large fixed cost per argument buffer, so a single blob argument
is much cheaper to launch than nine separate tensors. f32 regions (biases)
are stored bit-exact inside the f16 blob and read back via bitcast views.

Self-contained: hardcodes all shapes; host-side prep only reshapes/transposes
numpy arrays (sharding + time-major packing of x, weight transposes).
"""

import numpy as np

import bass_rust
import concourse.bass as bass
import concourse.mybir as mybir
from concourse.tile import TileContext
from concourse.vector_clock import ScopedClock
from concourse.bass_utils import run_bass_kernel_spmd

N_CORES = 8
B_TOT = 4096
L = 128          # sequence length (= 2*1024/16)
D = 16           # per-step input features
DA = 17          # + ones row (bias folding for layer 1)
H = 128          # hidden
G3 = 3 * H       # 384 stacked gates (r, z, n)
BL = B_TOT // N_CORES  # 512 batch per core
NCLS = 24
CHUNK = 8        # time steps of x staged into SBUF per DMA

F32 = mybir.dt.float32
F32R = mybir.dt.float32r
F16 = mybir.dt.float16
BF16 = mybir.dt.bfloat16
AF = mybir.ActivationFunctionType
OP = mybir.AluOpType

# Blob row layout ([NROWS, 512] f16 per core; f32 regions via bitcast->
# [NROWS, 256] f32 view, bit-exact).
ROW_XT = 0                     # 128*17 rows, [t,d] t-major, full 512 cols
ROW_L1W = ROW_XT + L * DA      # 17 rows, cols 0:384
ROW_HH1 = ROW_L1W + DA         # 128 rows, cols 0:384
ROW_IH2 = ROW_HH1 + H          # 128 rows, cols 0:384
ROW_HH2 = ROW_IH2 + H          # 128 rows, cols 0:384
ROW_FCW = ROW_HH2 + H          # 128 rows, cols 0:24
ROW_IDENT = ROW_FCW + H        # 128 rows, cols 0:128
ROW_BVEC = ROW_IDENT + H       # 128 rows, f32 cols 0:5
ROW_FCB = ROW_BVEC + H         # 24 rows, f32 col 0
NROWS = ROW_FCB + NCLS + 1     # 2986

# Tunables (grid-searched via TimelineSim, validated on HW).
CONFIG = {
    "pre_n_pe": True,    # accumulate t2 into P_x via PE identity matmul
    "split_rz1": True,  # separate r/z sigmoids for layer 1
    "d_eng": "v",        # engine for d = h - n
    "e_eng": "v",        # engine for e = z * d
    "h_eng": "v",        # engine for h' = n + e
}

DT = F16


class SplitDrainTileContext(TileContext):
    """Walrus (CoreV3) rejects instructions carrying >2 sync waits; Tile's
    kernel-tail drain accumulates one wait per outstanding engine/DMA-queue
    sem. Split them across a chain of drains (1 wait each)."""

    def _drain_and_barrier(self, tick_clock, wait_clock):
        nc = self.nc
        drain_inst = nc.sync.drain()
        wait_clock.add_sem_waits(
            drain_inst.ins, ScopedClock({None: tick_clock.global_clock})
        )
        si = drain_inst.ins.sync_info
        if si is not None and len(si.on_wait) > 1:
            waits = list(si.on_wait)
            si.on_wait = waits[:1]
            for w in waits[1:]:
                d2 = nc.sync.drain()
                d2.ins.sync_info = bass_rust.SyncInfo(on_wait=[w], on_update=[])
        nc.all_engine_barrier()
        popped = nc._tile_sem_poison_stack.pop()
        assert popped is self._sem_poison
        nc.clear_and_free_semaphores(list(self.sems.allocated().values()))
        nc.all_engine_barrier()


def _split_excess_waits(nc: bass.Bass, max_waits: int = 1) -> None:
    """Walrus (CoreV3 setupSyncWait) accepts at most 2 sem waits per
    instruction; Tile occasionally attaches 3+. Hoist the excess onto
    EventSemaphore instructions inserted right before the offender on the
    same engine (serial waits AND together)."""
    n = 0
    for fn in nc.m.functions:
        for bb in fn.blocks:
            out = []
            dirty = False
            for inst in bb.instructions:
                si = inst.sync_info
                if si is not None and len(si.on_wait) > max_waits:
                    waits = list(si.on_wait)
                    extra = waits[: len(waits) - max_waits]
                    for w in extra:
                        ev = mybir.InstEventSemaphore(
                            name=f"evs-waitsplit-{n}", ins=[], outs=[]
                        )
                        n += 1
                        ev.engine = inst.engine
                        ev.sync_info = bass_rust.SyncInfo(
                            on_wait=[w], on_update=[]
                        )
                        out.append(ev)
                    si.on_wait = waits[len(waits) - max_waits :]
                    dirty = True
                out.append(inst)
            if dirty:
                bb.instructions = out


def build_program(for_sim: bool = False, n_steps: int = L) -> bass.Bass:
    # for_sim: skip the walrus wait-limit workarounds (post-hoc IR mutations
    # that CoreSim's bookkeeping doesn't understand); semantics identical.
    nc = bass.Bass()

    blob_d = nc.declare_dram_parameter("blob", [NROWS, 512], DT, isOutput=False)
    b32 = blob_d.bitcast(F32)  # [NROWS, 256] f32 view of the same bytes
    # f16 output: the per-call donated-zero upload for the output buffer is
    # paid per dispatch at ~50 MB/s, so halving output bytes saves ~4 ms.
    out_d = nc.declare_dram_parameter("outT", [NCLS, BL], F16, isOutput=True)

    tc_cls = TileContext if for_sim else SplitDrainTileContext
    with tc_cls(nc) as tc:
        with (
            tc.tile_pool(name="singles", bufs=1) as singles,
            tc.tile_pool(name="xchunks", bufs=3) as xpool,
            tc.tile_pool(name="hstate", bufs=2) as hpool,
            tc.tile_pool(name="work", bufs=3) as work,
            tc.tile_pool(name="prz", bufs=1, space="PSUM") as przpool,
            tc.tile_pool(name="pnx", bufs=1, space="PSUM") as pnxpool,
        ):
            # --- constant loads -------------------------------------------
            l1w = singles.tile([DA, G3], DT, tag="l1w")
            hh1w = singles.tile([H, G3], DT, tag="hh1w")
            ih2w = singles.tile([H, G3], DT, tag="ih2w")
            hh2w = singles.tile([H, G3], DT, tag="hh2w")
            sbias = singles.tile([H, 5], F32, tag="sbias")
            fcw = singles.tile([H, NCLS], DT, tag="fcw")
            fcb = singles.tile([NCLS, 1], F32, tag="fcb")
            ident = singles.tile([H, H], DT, tag="ident")
            nc.sync.dma_start(out=ident[:], in_=blob_d[ROW_IDENT : ROW_IDENT + H, 0:H])
            nc.sync.dma_start(out=l1w[:], in_=blob_d[ROW_L1W : ROW_L1W + DA, 0:G3])
            nc.sync.dma_start(out=hh1w[:], in_=blob_d[ROW_HH1 : ROW_HH1 + H, 0:G3])
            nc.sync.dma_start(out=ih2w[:], in_=blob_d[ROW_IH2 : ROW_IH2 + H, 0:G3])
            nc.sync.dma_start(out=hh2w[:], in_=blob_d[ROW_HH2 : ROW_HH2 + H, 0:G3])
            nc.sync.dma_start(out=sbias[:], in_=b32[ROW_BVEC : ROW_BVEC + H, 0:5])
            nc.sync.dma_start(out=fcw[:], in_=blob_d[ROW_FCW : ROW_FCW + H, 0:NCLS])
            nc.sync.dma_start(out=fcb[:], in_=b32[ROW_FCB : ROW_FCB + NCLS, 0:1])

            ENG = {"v": nc.vector, "g": nc.gpsimd}

            def cell(tag, h_prev, x_rhs, xw, hw, rz_bias, n_hh_bias, n_ih_bias):
                """One GRU cell step, transposed layout [H partitions, BL free].

                h_prev: [H, BL] DT tile or None (t=0 => h=0, recurrent
                matmuls skipped). x_rhs: [K, BL] DT rhs for the input
                projection with lhsT xw [K, G3]; hw: [H, G3] recurrent lhsT.
                rz_bias: None (folded into xw) or (r_bias_ap, z_bias_ap).
                Returns the new [H, BL] DT hidden tile.
                """
                prz = przpool.tile([H, 2 * BL], F32, tag=f"prz{tag}")
                pn = pnxpool.tile([H, BL], F32, tag=f"pn{tag}")
                px = pnxpool.tile([H, BL], F32, tag=f"px{tag}")
                nc.tensor.matmul(prz[:, 0:BL], xw[:, 0:H], x_rhs,
                                 start=True, stop=h_prev is None)
                nc.tensor.matmul(prz[:, BL:], xw[:, H : 2 * H], x_rhs,
                                 start=True, stop=h_prev is None)
                if h_prev is not None:
                    nc.tensor.matmul(prz[:, 0:BL], hw[:, 0:H], h_prev[:],
                                     start=False, stop=True)
                    nc.tensor.matmul(prz[:, BL:], hw[:, H : 2 * H], h_prev[:],
                                     start=False, stop=True)
                nc.tensor.matmul(px[:], xw[:, 2 * H :], x_rhs, start=True,
                                 stop=not CONFIG["pre_n_pe"])
                if h_prev is not None:
                    nc.tensor.matmul(pn[:], hw[:, 2 * H :], h_prev[:],
                                     start=True, stop=True)

                split = rz_bias is not None or CONFIG["split_rz1"]
                if not split:
                    rz = work.tile([H, 2 * BL], DT, tag=f"rz{tag}")
                    nc.scalar.activation(rz[:], prz[:], AF.Sigmoid)
                    r, z = rz[:, 0:BL], rz[:, BL:]
                else:
                    rb = dict(bias=rz_bias[0]) if rz_bias else {}
                    zb = dict(bias=rz_bias[1]) if rz_bias else {}
                    rt = work.tile([H, BL], DT, tag=f"r{tag}")
                    nc.scalar.activation(rt[:], prz[:, 0:BL], AF.Sigmoid, **rb)
                    zt = work.tile([H, BL], DT, tag=f"z{tag}")
                    nc.scalar.activation(zt[:], prz[:, BL:], AF.Sigmoid, **zb)
                    r, z = rt[:], zt[:]

                t2 = work.tile([H, BL], DT, tag=f"t2{tag}")
                if h_prev is not None:
                    # t2 = (hn + b_hh_n) * r
                    nc.vector.scalar_tensor_tensor(
                        t2[:], pn[:], n_hh_bias, r, op0=OP.add, op1=OP.mult
                    )
                else:
                    nc.vector.tensor_scalar_mul(t2[:], r, n_hh_bias)
                n = work.tile([H, BL], DT, tag=f"n{tag}")
                nb = dict(bias=n_ih_bias) if n_ih_bias is not None else {}
                if CONFIG["pre_n_pe"]:
                    # px += I.T @ t2 on the PE, then tanh straight off PSUM
                    nc.tensor.matmul(px[:], ident[:], t2[:], start=False, stop=True)
                    nc.scalar.activation(n[:], px[:], AF.Tanh, **nb)
                else:
                    pre = work.tile([H, BL], F32, tag=f"pre{tag}")
                    nc.vector.tensor_add(pre[:], t2[:], px[:])
                    nc.scalar.activation(n[:], pre[:], AF.Tanh, **nb)
                d = work.tile([H, BL], DT, tag=f"d{tag}")
                if h_prev is not None:
                    ENG[CONFIG["d_eng"]].tensor_sub(d[:], h_prev[:], n[:])
                else:
                    ENG[CONFIG["d_eng"]].tensor_scalar_mul(d[:], n[:], -1.0)
                e = work.tile([H, BL], DT, tag=f"e{tag}")
                ENG[CONFIG["e_eng"]].tensor_mul(e[:], z, d[:])
                h_new = hpool.tile([H, BL], DT, tag=f"h{tag}")
                ENG[CONFIG["h_eng"]].tensor_add(h_new[:], n[:], e[:])
                return h_new

            h1 = None
            h2 = None
            xc = None
            for t in range(n_steps):
                if t % CHUNK == 0:
                    xc = xpool.tile([DA, CHUNK, BL], DT, tag="xc")
                    nc.sync.dma_start(
                        out=xc[:],
                        in_=blob_d[t * DA : (t + CHUNK) * DA].rearrange(
                            "(t d) b -> d t b", t=CHUNK
                        ),
                    )
                xg = xc[:, t % CHUNK, :]
                h1 = cell("1", h1, xg, l1w, hh1w, None, sbias[:, 0:1], None)
                h2 = cell("2", h2, h1[:], ih2w, hh2w,
                          (sbias[:, 1:2], sbias[:, 2:3]), sbias[:, 3:4],
                          sbias[:, 4:5])

            # ---------------- FC head ------------------------------------
            pfc = pnxpool.tile([NCLS, BL], F32, tag="pn1")
            nc.tensor.matmul(pfc[:], fcw[:], h2[:], start=True, stop=True)
            outs = work.tile([NCLS, BL], F16, tag="outs")
            nc.scalar.activation(outs[:], pfc[:], AF.Identity, bias=fcb[:])
            nc.sync.dma_start(out=out_d[:], in_=outs[:])

    if not for_sim:
        _split_excess_waits(nc)
    return nc


def prep_in_maps(inputs: dict) -> list[dict]:
    """Shard + repack the full-problem numpy inputs into per-core blobs."""
    x = np.ascontiguousarray(np.asarray(inputs["x"], dtype=np.float32))
    w_ih1 = np.asarray(inputs["w_ih1"], np.float32)
    w_hh1 = np.asarray(inputs["w_hh1"], np.float32)
    b_ih1 = np.asarray(inputs["b_ih1"], np.float32)
    b_hh1 = np.asarray(inputs["b_hh1"], np.float32)
    w_ih2 = np.asarray(inputs["w_ih2"], np.float32)
    w_hh2 = np.asarray(inputs["w_hh2"], np.float32)
    b_ih2 = np.asarray(inputs["b_ih2"], np.float32)
    b_hh2 = np.asarray(inputs["b_hh2"], np.float32)
    fc_w = np.asarray(inputs["fc_w"], np.float32)
    fc_b = np.asarray(inputs["fc_b"], np.float32)

    blob = np.zeros((N_CORES, NROWS, 512), np.float16)

    # x: (4096, 2, 1024) -> per-core time-major transposed rows [t,d] -> b
    xr = x.reshape(N_CORES, BL, 2, L, D // 2)  # [core, b, ch, t, j]
    bxT = blob[:, ROW_XT : ROW_XT + L * DA, :].reshape(N_CORES, L, DA, BL)
    bxT[:, :, 0 : D // 2, :] = xr[:, :, 0].transpose(0, 2, 3, 1)
    bxT[:, :, D // 2 : D, :] = xr[:, :, 1].transpose(0, 2, 3, 1)
    bxT[:, :, D, :] = 1.0  # ones row: folds layer-1 biases into the matmul

    # layer-1 combined input-proj weights + bias row.
    # r/z columns carry b_ih1+b_hh1; n columns carry b_ih1 only (b_hh1_n must
    # be applied inside r*(hn+b_hh1_n)).
    l1w = np.empty((DA, G3), np.float32)
    l1w[0:D, :] = w_ih1.T
    bias_row = b_ih1.copy()
    bias_row[0 : 2 * H] += b_hh1[0 : 2 * H]
    l1w[D, :] = bias_row

    bvec = np.stack(
        [
            b_hh1[2 * H : 3 * H],                     # col 0: L1 n-gate hh bias
            (b_ih2 + b_hh2)[0:H],                     # col 1: L2 r bias
            (b_ih2 + b_hh2)[H : 2 * H],               # col 2: L2 z bias
            b_hh2[2 * H : 3 * H],                     # col 3: L2 n-gate hh bias
            b_ih2[2 * H : 3 * H],                     # col 4: L2 n-gate ih bias
        ],
        axis=1,
    ).astype(np.float32)

    blob[:, ROW_L1W : ROW_L1W + DA, 0:G3] = l1w.astype(np.float16)
    blob[:, ROW_HH1 : ROW_HH1 + H, 0:G3] = w_hh1.T.astype(np.float16)
    blob[:, ROW_IH2 : ROW_IH2 + H, 0:G3] = w_ih2.T.astype(np.float16)
    blob[:, ROW_HH2 : ROW_HH2 + H, 0:G3] = w_hh2.T.astype(np.float16)
    blob[:, ROW_FCW : ROW_FCW + H, 0:NCLS] = fc_w.T.astype(np.float16)
    blob[:, ROW_IDENT : ROW_IDENT + H, 0:H] = np.eye(H, dtype=np.float16)
    # f32 regions, stored bit-exact as pairs of f16 slots
    blob[:, ROW_BVEC : ROW_BVEC + H, 0:10] = bvec.view(np.float16)
    blob[:, ROW_FCB : ROW_FCB + NCLS, 0:2] = fc_b[:, None].view(np.float16)

    return [{"blob": blob[c]} for c in range(N_CORES)]


def assemble_output(results: list[dict]) -> np.ndarray:
    # per-core f16 outT [24, BL] -> (4096, 24) f32
    return np.concatenate([r["outT"].T for r in results], axis=0).astype(np.float32)


_NC_CACHE = None
_EXEC_CACHE = None


def bass_io_names(nc: bass.Bass):
    """(in_names, out_names, out_avals) from the program's allocations."""
    import jax

    in_names, out_names, out_avals = [], [], []
    for alloc in nc.m.functions[0].allocations:
        if not isinstance(alloc, mybir.MemoryLocationSet):
            continue
        name = alloc.memorylocations[0].name
        if alloc.kind == "ExternalInput":
            in_names.append(name)
        elif alloc.kind == "ExternalOutput":
            out_avals.append(
                jax.core.ShapedArray(
                    tuple(alloc.tensor_shape), mybir.dt.np(alloc.dtype)
                )
            )
            out_names.append(name)
    return in_names, out_names, out_avals


def build_executor(nc: bass.Bass):
    """Jitted 8-core dispatch of the prebuilt program.

    Leaner than bass_utils.run_bass_kernel_spmd's per-call path: the jit is
    built once and reused, and no donated zero output buffers are passed.
    (With empty input/output aliases the bass_exec lowering allocates fresh
    device buffers for outputs; the zero operands only exist to pre-zero
    outputs for kernels that don't write every element — this kernel writes
    all of outT, so they'd be dead weight uploaded on every call.)
    """
    import jax
    from jax.experimental.shard_map import shard_map
    from jax.sharding import Mesh, PartitionSpec
    from concourse import bass2jax

    bass2jax.install_neuronx_cc_hook()
    assert nc.dbg_addr is None
    partition_name = nc.partition_id_tensor.name if nc.partition_id_tensor else None
    in_names, out_names, out_avals = bass_io_names(nc)
    in_names = [nm for nm in in_names if nm != partition_name]
    all_in_names = list(in_names) + ([partition_name] if partition_name else [])

    def _body(*args):
        operands = list(args)
        if partition_name is not None:
            operands.append(bass2jax.partition_id_tensor())
        outs = bass2jax._bass_exec_p.bind(
            *operands,
            out_avals=tuple(out_avals),
            in_names=tuple(all_in_names),
            out_names=tuple(out_names),
            lowering_input_output_aliases=(),
            sim_require_finite=True,
            sim_require_nnan=True,
            nc=nc,
        )
        return tuple(outs)

    devices = jax.devices()[:N_CORES]
    mesh = Mesh(np.asarray(devices), ("core",))
    sharded = jax.jit(
        shard_map(
            _body,
            mesh=mesh,
            in_specs=(PartitionSpec("core"),) * len(in_names),
            out_specs=(PartitionSpec("core"),) * len(out_names),
            check_rep=False,
        )
    )
    return sharded, in_names, out_names


def kernel(**inputs) -> np.ndarray:
    global _NC_CACHE, _EXEC_CACHE
    if _NC_CACHE is None:
        _NC_CACHE = build_program()
    in_maps = prep_in_maps(inputs)
    try:
        if _EXEC_CACHE is None:
            _EXEC_CACHE = build_executor(_NC_CACHE)
        sharded, in_names, out_names = _EXEC_CACHE
        concat_in = [
            np.concatenate([m[nm] for m in in_maps], axis=0) for nm in in_names
        ]
        outs = [np.asarray(o) for o in sharded(*concat_in)]
        results = [
            {
                nm: outs[i].reshape(
                    N_CORES, outs[i].shape[0] // N_CORES, *outs[i].shape[1:]
                )[c]
                for i, nm in enumerate(out_names)
            }
            for c in range(N_CORES)
        ]
        return assemble_output(results)
    except Exception:
        import traceback

        traceback.print_exc()
        print("kernel: fast dispatch failed; falling back to run_bass_kernel_spmd")
        res = run_bass_kernel_spmd(_NC_CACHE, in_maps, list(range(N_CORES)))
        return assemble_output(res.results)


# revision 9
# speedup vs baseline: 7.4788x; 2.8061x over previous
"""Two-layer GRU (16->128->128) + FC(128->24) head on 8 Trainium2 NeuronCores.

Strategy: data-parallel over the batch (4096 -> 512 per core); tiny weights
replicated. On each core the hidden state lives transposed in SBUF as
[H=128 partitions, B=512 free]. Per time step, gate pre-activations are
accumulated in PSUM by fp16 matmuls (input-projection + recurrent + biases
folded in), sigmoid/tanh run on the scalar engine with per-partition bias
APs, and the cell update is spread across vector + gpsimd engines.

All per-core inputs (time-major packed x, weights, biases, identity) are
packed into ONE [NROWS, 512] f16 DRAM tensor per core: the PJRT dispatch
path pays a large fixed cost per argument buffer, so a single blob argument
is much cheaper to launch than nine separate tensors. f32 regions (biases)
are stored bit-exact inside the f16 blob and read back via bitcast views.

Self-contained: hardcodes all shapes; host-side prep only reshapes/transposes
numpy arrays (sharding + time-major packing of x, weight transposes).
"""

import numpy as np

import bass_rust
import concourse.bass as bass
import concourse.mybir as mybir
from concourse.tile import TileContext
from concourse.vector_clock import ScopedClock
from concourse.bass_utils import run_bass_kernel_spmd

N_CORES = 8
B_TOT = 4096
L = 128          # sequence length (= 2*1024/16)
D = 16           # per-step input features
DA = 17          # + ones row (bias folding for layer 1)
H = 128          # hidden
G3 = 3 * H       # 384 stacked gates (r, z, n)
BL = B_TOT // N_CORES  # 512 batch per core
NCLS = 24
CHUNK = 8        # time steps of x staged into SBUF per DMA

F32 = mybir.dt.float32
F32R = mybir.dt.float32r
F16 = mybir.dt.float16
BF16 = mybir.dt.bfloat16
AF = mybir.ActivationFunctionType
OP = mybir.AluOpType

# Blob row layout ([NROWS, 512] f16 per core; f32 regions via bitcast->
# [NROWS, 256] f32 view, bit-exact).
ROW_XT = 0                     # 128*17 rows, [t,d] t-major, full 512 cols
ROW_L1W = ROW_XT + L * DA      # 17 rows, cols 0:384
ROW_HH1 = ROW_L1W + DA         # 128 rows, cols 0:384
ROW_IH2 = ROW_HH1 + H          # 128 rows, cols 0:384
ROW_HH2 = ROW_IH2 + H          # 128 rows, cols 0:384
ROW_FCW = ROW_HH2 + H          # 128 rows, cols 0:24
ROW_IDENT = ROW_FCW + H        # 128 rows, cols 0:128
ROW_BVEC = ROW_IDENT + H       # 128 rows, f32 cols 0:5
ROW_FCB = ROW_BVEC + H         # 24 rows, f32 col 0
NROWS = ROW_FCB + NCLS + 1     # 2986

# Tunables (grid-searched via TimelineSim, validated on HW).
CONFIG = {
    "pre_n_pe": True,    # accumulate t2 into P_x via PE identity matmul
    "split_rz1": True,  # separate r/z sigmoids for layer 1
    "d_eng": "v",        # engine for d = h - n
    "e_eng": "v",        # engine for e = z * d
    "h_eng": "v",        # engine for h' = n + e
}

DT = F16


class SplitDrainTileContext(TileContext):
    """Walrus (CoreV3) rejects instructions carrying >2 sync waits; Tile's
    kernel-tail drain accumulates one wait per outstanding engine/DMA-queue
    sem. Split them across a chain of drains (1 wait each)."""

    def _drain_and_barrier(self, tick_clock, wait_clock):
        nc = self.nc
        drain_inst = nc.sync.drain()
        wait_clock.add_sem_waits(
            drain_inst.ins, ScopedClock({None: tick_clock.global_clock})
        )
        si = drain_inst.ins.sync_info
        if si is not None and len(si.on_wait) > 1:
            waits = list(si.on_wait)
            si.on_wait = waits[:1]
            for w in waits[1:]:
                d2 = nc.sync.drain()
                d2.ins.sync_info = bass_rust.SyncInfo(on_wait=[w], on_update=[])
        nc.all_engine_barrier()
        popped = nc._tile_sem_poison_stack.pop()
        assert popped is self._sem_poison
        nc.clear_and_free_semaphores(list(self.sems.allocated().values()))
        nc.all_engine_barrier()


def _split_excess_waits(nc: bass.Bass, max_waits: int = 1) -> None:
    """Walrus (CoreV3 setupSyncWait) accepts at most 2 sem waits per
    instruction; Tile occasionally attaches 3+. Hoist the excess onto
    EventSemaphore instructions inserted right before the offender on the
    same engine (serial waits AND together)."""
    n = 0
    for fn in nc.m.functions:
        for bb in fn.blocks:
            out = []
            dirty = False
            for inst in bb.instructions:
                si = inst.sync_info
                if si is not None and len(si.on_wait) > max_waits:
                    waits = list(si.on_wait)
                    extra = waits[: len(waits) - max_waits]
                    for w in extra:
                        ev = mybir.InstEventSemaphore(
                            name=f"evs-waitsplit-{n}", ins=[], outs=[]
                        )
                        n += 1
                        ev.engine = inst.engine
                        ev.sync_info = bass_rust.SyncInfo(
                            on_wait=[w], on_update=[]
                        )
                        out.append(ev)
                    si.on_wait = waits[len(waits) - max_waits :]
                    dirty = True
                out.append(inst)
            if dirty:
                bb.instructions = out


def build_program(for_sim: bool = False, n_steps: int = L) -> bass.Bass:
    # for_sim: skip the walrus wait-limit workarounds (post-hoc IR mutations
    # that CoreSim's bookkeeping doesn't understand); semantics identical.
    nc = bass.Bass()

    blob_d = nc.declare_dram_parameter("blob", [NROWS, 512], DT, isOutput=False)
    b32 = blob_d.bitcast(F32)  # [NROWS, 256] f32 view of the same bytes
    # f16 output: the per-call donated-zero upload for the output buffer is
    # paid per dispatch at ~50 MB/s, so halving output bytes saves ~4 ms.
    out_d = nc.declare_dram_parameter("outT", [NCLS, BL], F16, isOutput=True)

    tc_cls = TileContext if for_sim else SplitDrainTileContext
    with tc_cls(nc) as tc:
        with (
            tc.tile_pool(name="singles", bufs=1) as singles,
            tc.tile_pool(name="xchunks", bufs=3) as xpool,
            tc.tile_pool(name="hstate", bufs=2) as hpool,
            tc.tile_pool(name="work", bufs=3) as work,
            tc.tile_pool(name="prz", bufs=1, space="PSUM") as przpool,
            tc.tile_pool(name="pnx", bufs=1, space="PSUM") as pnxpool,
        ):
            # --- constant loads -------------------------------------------
            l1w = singles.tile([DA, G3], DT, tag="l1w")
            hh1w = singles.tile([H, G3], DT, tag="hh1w")
            ih2w = singles.tile([H, G3], DT, tag="ih2w")
            hh2w = singles.tile([H, G3], DT, tag="hh2w")
            sbias = singles.tile([H, 5], F32, tag="sbias")
            fcw = singles.tile([H, NCLS], DT, tag="fcw")
            fcb = singles.tile([NCLS, 1], F32, tag="fcb")
            ident = singles.tile([H, H], DT, tag="ident")
            nc.sync.dma_start(out=ident[:], in_=blob_d[ROW_IDENT : ROW_IDENT + H, 0:H])
            nc.sync.dma_start(out=l1w[:], in_=blob_d[ROW_L1W : ROW_L1W + DA, 0:G3])
            nc.sync.dma_start(out=hh1w[:], in_=blob_d[ROW_HH1 : ROW_HH1 + H, 0:G3])
            nc.sync.dma_start(out=ih2w[:], in_=blob_d[ROW_IH2 : ROW_IH2 + H, 0:G3])
            nc.sync.dma_start(out=hh2w[:], in_=blob_d[ROW_HH2 : ROW_HH2 + H, 0:G3])
            nc.sync.dma_start(out=sbias[:], in_=b32[ROW_BVEC : ROW_BVEC + H, 0:5])
            nc.sync.dma_start(out=fcw[:], in_=blob_d[ROW_FCW : ROW_FCW + H, 0:NCLS])
            nc.sync.dma_start(out=fcb[:], in_=b32[ROW_FCB : ROW_FCB + NCLS, 0:1])

            ENG = {"v": nc.vector, "g": nc.gpsimd}

            def cell(tag, h_prev, x_rhs, xw, hw, rz_bias, n_hh_bias, n_ih_bias):
                """One GRU cell step, transposed layout [H partitions, BL free].

                h_prev: [H, BL] DT tile or None (t=0 => h=0, recurrent
                matmuls skipped). x_rhs: [K, BL] DT rhs for the input
                projection with lhsT xw [K, G3]; hw: [H, G3] recurrent lhsT.
                rz_bias: None (folded into xw) or (r_bias_ap, z_bias_ap).
                Returns the new [H, BL] DT hidden tile.
                """
                prz = przpool.tile([H, 2 * BL], F32, tag=f"prz{tag}")
                pn = pnxpool.tile([H, BL], F32, tag=f"pn{tag}")
                px = pnxpool.tile([H, BL], F32, tag=f"px{tag}")
                nc.tensor.matmul(prz[:, 0:BL], xw[:, 0:H], x_rhs,
                                 start=True, stop=h_prev is None)
                nc.tensor.matmul(prz[:, BL:], xw[:, H : 2 * H], x_rhs,
                                 start=True, stop=h_prev is None)
                if h_prev is not None:
                    nc.tensor.matmul(prz[:, 0:BL], hw[:, 0:H], h_prev[:],
                                     start=False, stop=True)
                    nc.tensor.matmul(prz[:, BL:], hw[:, H : 2 * H], h_prev[:],
                                     start=False, stop=True)
                nc.tensor.matmul(px[:], xw[:, 2 * H :], x_rhs, start=True,
                                 stop=not CONFIG["pre_n_pe"])
                if h_prev is not None:
                    nc.tensor.matmul(pn[:], hw[:, 2 * H :], h_prev[:],
                                     start=True, stop=True)

                split = rz_bias is not None or CONFIG["split_rz1"]
                if not split:
                    rz = work.tile([H, 2 * BL], DT, tag=f"rz{tag}")
                    nc.scalar.activation(rz[:], prz[:], AF.Sigmoid)
                    r, z = rz[:, 0:BL], rz[:, BL:]
                else:
                    rb = dict(bias=rz_bias[0]) if rz_bias else {}
                    zb = dict(bias=rz_bias[1]) if rz_bias else {}
                    rt = work.tile([H, BL], DT, tag=f"r{tag}")
                    nc.scalar.activation(rt[:], prz[:, 0:BL], AF.Sigmoid, **rb)
                    zt = work.tile([H, BL], DT, tag=f"z{tag}")
                    nc.scalar.activation(zt[:], prz[:, BL:], AF.Sigmoid, **zb)
                    r, z = rt[:], zt[:]

                t2 = work.tile([H, BL], DT, tag=f"t2{tag}")
                if h_prev is not None:
                    # t2 = (hn + b_hh_n) * r
                    nc.vector.scalar_tensor_tensor(
                        t2[:], pn[:], n_hh_bias, r, op0=OP.add, op1=OP.mult
                    )
                else:
                    nc.vector.tensor_scalar_mul(t2[:], r, n_hh_bias)
                n = work.tile([H, BL], DT, tag=f"n{tag}")
                nb = dict(bias=n_ih_bias) if n_ih_bias is not None else {}
                if CONFIG["pre_n_pe"]:
                    # px += I.T @ t2 on the PE, then tanh straight off PSUM
                    nc.tensor.matmul(px[:], ident[:], t2[:], start=False, stop=True)
                    nc.scalar.activation(n[:], px[:], AF.Tanh, **nb)
                else:
                    pre = work.tile([H, BL], F32, tag=f"pre{tag}")
                    nc.vector.tensor_add(pre[:], t2[:], px[:])
                    nc.scalar.activation(n[:], pre[:], AF.Tanh, **nb)
                d = work.tile([H, BL], DT, tag=f"d{tag}")
                if h_prev is not None:
                    ENG[CONFIG["d_eng"]].tensor_sub(d[:], h_prev[:], n[:])
                else:
                    ENG[CONFIG["d_eng"]].tensor_scalar_mul(d[:], n[:], -1.0)
                e = work.tile([H, BL], DT, tag=f"e{tag}")
                ENG[CONFIG["e_eng"]].tensor_mul(e[:], z, d[:])
                h_new = hpool.tile([H, BL], DT, tag=f"h{tag}")
                ENG[CONFIG["h_eng"]].tensor_add(h_new[:], n[:], e[:])
                return h_new

            h1 = None
            h2 = None
            xc = None
            for t in range(n_steps):
                if t % CHUNK == 0:
                    tw = t % L  # wrap: lets timing builds run n_steps > L
                    xc = xpool.tile([DA, CHUNK, BL], DT, tag="xc")
                    nc.sync.dma_start(
                        out=xc[:],
                        in_=blob_d[tw * DA : (tw + CHUNK) * DA].rearrange(
                            "(t d) b -> d t b", t=CHUNK
                        ),
                    )
                xg = xc[:, t % CHUNK, :]
                h1 = cell("1", h1, xg, l1w, hh1w, None, sbias[:, 0:1], None)
                h2 = cell("2", h2, h1[:], ih2w, hh2w,
                          (sbias[:, 1:2], sbias[:, 2:3]), sbias[:, 3:4],
                          sbias[:, 4:5])

            # ---------------- FC head ------------------------------------
            pfc = pnxpool.tile([NCLS, BL], F32, tag="pn1")
            nc.tensor.matmul(pfc[:], fcw[:], h2[:], start=True, stop=True)
            outs = work.tile([NCLS, BL], F16, tag="outs")
            nc.scalar.activation(outs[:], pfc[:], AF.Identity, bias=fcb[:])
            nc.sync.dma_start(out=out_d[:], in_=outs[:])

    if not for_sim:
        _split_excess_waits(nc)
    return nc


def prep_in_maps(inputs: dict) -> list[dict]:
    """Shard + repack the full-problem numpy inputs into per-core blobs."""
    x = np.ascontiguousarray(np.asarray(inputs["x"], dtype=np.float32))
    w_ih1 = np.asarray(inputs["w_ih1"], np.float32)
    w_hh1 = np.asarray(inputs["w_hh1"], np.float32)
    b_ih1 = np.asarray(inputs["b_ih1"], np.float32)
    b_hh1 = np.asarray(inputs["b_hh1"], np.float32)
    w_ih2 = np.asarray(inputs["w_ih2"], np.float32)
    w_hh2 = np.asarray(inputs["w_hh2"], np.float32)
    b_ih2 = np.asarray(inputs["b_ih2"], np.float32)
    b_hh2 = np.asarray(inputs["b_hh2"], np.float32)
    fc_w = np.asarray(inputs["fc_w"], np.float32)
    fc_b = np.asarray(inputs["fc_b"], np.float32)

    blob = np.zeros((N_CORES, NROWS, 512), np.float16)

    # x: (4096, 2, 1024) -> per-core time-major transposed rows [t,d] -> b
    xr = x.reshape(N_CORES, BL, 2, L, D // 2)  # [core, b, ch, t, j]
    bxT = blob[:, ROW_XT : ROW_XT + L * DA, :].reshape(N_CORES, L, DA, BL)
    bxT[:, :, 0 : D // 2, :] = xr[:, :, 0].transpose(0, 2, 3, 1)
    bxT[:, :, D // 2 : D, :] = xr[:, :, 1].transpose(0, 2, 3, 1)
    bxT[:, :, D, :] = 1.0  # ones row: folds layer-1 biases into the matmul

    # layer-1 combined input-proj weights + bias row.
    # r/z columns carry b_ih1+b_hh1; n columns carry b_ih1 only (b_hh1_n must
    # be applied inside r*(hn+b_hh1_n)).
    l1w = np.empty((DA, G3), np.float32)
    l1w[0:D, :] = w_ih1.T
    bias_row = b_ih1.copy()
    bias_row[0 : 2 * H] += b_hh1[0 : 2 * H]
    l1w[D, :] = bias_row

    bvec = np.stack(
        [
            b_hh1[2 * H : 3 * H],                     # col 0: L1 n-gate hh bias
            (b_ih2 + b_hh2)[0:H],                     # col 1: L2 r bias
            (b_ih2 + b_hh2)[H : 2 * H],               # col 2: L2 z bias
            b_hh2[2 * H : 3 * H],                     # col 3: L2 n-gate hh bias
            b_ih2[2 * H : 3 * H],                     # col 4: L2 n-gate ih bias
        ],
        axis=1,
    ).astype(np.float32)

    blob[:, ROW_L1W : ROW_L1W + DA, 0:G3] = l1w.astype(np.float16)
    blob[:, ROW_HH1 : ROW_HH1 + H, 0:G3] = w_hh1.T.astype(np.float16)
    blob[:, ROW_IH2 : ROW_IH2 + H, 0:G3] = w_ih2.T.astype(np.float16)
    blob[:, ROW_HH2 : ROW_HH2 + H, 0:G3] = w_hh2.T.astype(np.float16)
    blob[:, ROW_FCW : ROW_FCW + H, 0:NCLS] = fc_w.T.astype(np.float16)
    blob[:, ROW_IDENT : ROW_IDENT + H, 0:H] = np.eye(H, dtype=np.float16)
    # f32 regions, stored bit-exact as pairs of f16 slots
    blob[:, ROW_BVEC : ROW_BVEC + H, 0:10] = bvec.view(np.float16)
    blob[:, ROW_FCB : ROW_FCB + NCLS, 0:2] = fc_b[:, None].view(np.float16)

    return [{"blob": blob[c]} for c in range(N_CORES)]


def assemble_output(results: list[dict]) -> np.ndarray:
    # per-core f16 outT [24, BL] -> (4096, 24) f32
    return np.concatenate([r["outT"].T for r in results], axis=0).astype(np.float32)


_NC_CACHE = None
_EXEC_CACHE = None


def bass_io_names(nc: bass.Bass):
    """(in_names, out_names, out_avals) from the program's allocations."""
    import jax

    in_names, out_names, out_avals = [], [], []
    for alloc in nc.m.functions[0].allocations:
        if not isinstance(alloc, mybir.MemoryLocationSet):
            continue
        name = alloc.memorylocations[0].name
        if alloc.kind == "ExternalInput":
            in_names.append(name)
        elif alloc.kind == "ExternalOutput":
            out_avals.append(
                jax.core.ShapedArray(
                    tuple(alloc.tensor_shape), mybir.dt.np(alloc.dtype)
                )
            )
            out_names.append(name)
    return in_names, out_names, out_avals


def build_executor(nc: bass.Bass):
    """Jitted 8-core dispatch of the prebuilt program.

    Leaner than bass_utils.run_bass_kernel_spmd's per-call path: the jit is
    built once and reused, and no donated zero output buffers are passed.
    (With empty input/output aliases the bass_exec lowering allocates fresh
    device buffers for outputs; the zero operands only exist to pre-zero
    outputs for kernels that don't write every element — this kernel writes
    all of outT, so they'd be dead weight uploaded on every call.)
    """
    import jax
    from jax.experimental.shard_map import shard_map
    from jax.sharding import Mesh, PartitionSpec
    from concourse import bass2jax

    bass2jax.install_neuronx_cc_hook()
    assert nc.dbg_addr is None
    partition_name = nc.partition_id_tensor.name if nc.partition_id_tensor else None
    in_names, out_names, out_avals = bass_io_names(nc)
    in_names = [nm for nm in in_names if nm != partition_name]
    all_in_names = list(in_names) + ([partition_name] if partition_name else [])

    def _body(*args):
        operands = list(args)
        if partition_name is not None:
            operands.append(bass2jax.partition_id_tensor())
        outs = bass2jax._bass_exec_p.bind(
            *operands,
            out_avals=tuple(out_avals),
            in_names=tuple(all_in_names),
            out_names=tuple(out_names),
            lowering_input_output_aliases=(),
            sim_require_finite=True,
            sim_require_nnan=True,
            nc=nc,
        )
        return tuple(outs)

    devices = jax.devices()[:N_CORES]
    mesh = Mesh(np.asarray(devices), ("core",))
    sharded = jax.jit(
        shard_map(
            _body,
            mesh=mesh,
            in_specs=(PartitionSpec("core"),) * len(in_names),
            out_specs=(PartitionSpec("core"),) * len(out_names),
            check_rep=False,
        )
    )
    return sharded, in_names, out_names


def kernel(**inputs) -> np.ndarray:
    global _NC_CACHE, _EXEC_CACHE
    if _NC_CACHE is None:
        _NC_CACHE = build_program()
    in_maps = prep_in_maps(inputs)
    try:
        if _EXEC_CACHE is None:
            _EXEC_CACHE = build_executor(_NC_CACHE)
        sharded, in_names, out_names = _EXEC_CACHE
        concat_in = [
            np.concatenate([m[nm] for m in in_maps], axis=0) for nm in in_names
        ]
        outs = [np.asarray(o) for o in sharded(*concat_in)]
        results = [
            {
                nm: outs[i].reshape(
                    N_CORES, outs[i].shape[0] // N_CORES, *outs[i].shape[1:]
                )[c]
                for i, nm in enumerate(out_names)
            }
            for c in range(N_CORES)
        ]
        return assemble_output(results)
    except Exception:
        import traceback

        traceback.print_exc()
        print("kernel: fast dispatch failed; falling back to run_bass_kernel_spmd")
        res = run_bass_kernel_spmd(_NC_CACHE, in_maps, list(range(N_CORES)))
        return assemble_output(res.results)


# revision 10
# speedup vs baseline: 9.8563x; 1.3179x over previous
"""Two-layer GRU (16->128->128) + FC(128->24) head on 4 Trainium2 NeuronCores.

Strategy: data-parallel over the batch (4096 -> 1024 per core); tiny weights
replicated. 4 cores (not 8): the per-call dispatch cost of the axon PJRT
client scales with core count (~0.45 ms/core/call) and dominates device
execution, so halving the cores halves the launch cost while the doubled
per-core device time still hides under it.

On each core the hidden state lives transposed in SBUF as [H=128 partitions,
B=1024 free]. Per time step, gate pre-activations are accumulated in PSUM by
fp16 matmuls (input-projection + recurrent + biases folded in) — each split
into 2x512-column halves because a matmul output may span at most one 2 KiB
PSUM bank per partition. Both GRU layers share one PSUM tile set
(prz 4 banks + pn 2 + px 2 = all 8 banks). sigmoid/tanh run on the scalar
engine with per-partition bias APs; the cell update runs on the vector
engine over full [H, 1024] tiles.

All per-core inputs (time-major packed x, weights, biases, identity) are
packed into ONE [NROWS, 512] f16 DRAM tensor per core: the dispatch path
also pays a large fixed cost per argument buffer. f32 regions (biases) are
stored bit-exact inside the f16 blob and read back via bitcast views.

Self-contained: hardcodes all shapes; host-side prep only reshapes/transposes
numpy arrays (sharding + time-major packing of x, weight transposes).
"""

import numpy as np

import bass_rust
import concourse.bass as bass
import concourse.mybir as mybir
from concourse.tile import TileContext
from concourse.vector_clock import ScopedClock
from concourse.bass_utils import run_bass_kernel_spmd

N_CORES = 4
B_TOT = 4096
L = 128          # sequence length (= 2*1024/16)
D = 16           # per-step input features
DA = 17          # + ones row (bias folding for layer 1)
H = 128          # hidden
G3 = 3 * H       # 384 stacked gates (r, z, n)
BL = B_TOT // N_CORES  # 1024 batch per core
HB = 512         # batch half: max matmul-output columns per PSUM bank
NCLS = 24
CHUNK = 8        # time steps of x staged into SBUF per DMA

F32 = mybir.dt.float32
F16 = mybir.dt.float16
AF = mybir.ActivationFunctionType
OP = mybir.AluOpType

# Blob row layout ([NROWS, 512] f16 per core; f32 regions via bitcast->
# [NROWS, 256] f32 view, bit-exact). x region: two rows per (t, d), one per
# 512-wide batch half.
ROW_XT = 0                     # 128*17*2 rows, [t, d, half] t-major
ROW_L1W = ROW_XT + L * DA * 2  # 17 rows, cols 0:384
ROW_HH1 = ROW_L1W + DA         # 128 rows, cols 0:384
ROW_IH2 = ROW_HH1 + H          # 128 rows, cols 0:384
ROW_HH2 = ROW_IH2 + H          # 128 rows, cols 0:384
ROW_FCW = ROW_HH2 + H          # 128 rows, cols 0:24
ROW_IDENT = ROW_FCW + H        # 128 rows, cols 0:128
ROW_BVEC = ROW_IDENT + H       # 128 rows, f32 cols 0:5
ROW_FCB = ROW_BVEC + H         # 24 rows, f32 col 0
NROWS = ROW_FCB + NCLS + 1     # 5162

DT = F16


class SplitDrainTileContext(TileContext):
    """Walrus (CoreV3) rejects instructions carrying >2 sync waits; Tile's
    kernel-tail drain accumulates one wait per outstanding engine/DMA-queue
    sem. Split them across a chain of drains (1 wait each)."""

    def _drain_and_barrier(self, tick_clock, wait_clock):
        nc = self.nc
        drain_inst = nc.sync.drain()
        wait_clock.add_sem_waits(
            drain_inst.ins, ScopedClock({None: tick_clock.global_clock})
        )
        si = drain_inst.ins.sync_info
        if si is not None and len(si.on_wait) > 1:
            waits = list(si.on_wait)
            si.on_wait = waits[:1]
            for w in waits[1:]:
                d2 = nc.sync.drain()
                d2.ins.sync_info = bass_rust.SyncInfo(on_wait=[w], on_update=[])
        nc.all_engine_barrier()
        popped = nc._tile_sem_poison_stack.pop()
        assert popped is self._sem_poison
        nc.clear_and_free_semaphores(list(self.sems.allocated().values()))
        nc.all_engine_barrier()


def _split_excess_waits(nc: bass.Bass, max_waits: int = 1) -> None:
    """Walrus (CoreV3 setupSyncWait) accepts at most 2 sem waits per
    instruction; Tile occasionally attaches 3+. Hoist the excess onto
    EventSemaphore instructions inserted right before the offender on the
    same engine (serial waits AND together)."""
    n = 0
    for fn in nc.m.functions:
        for bb in fn.blocks:
            out = []
            dirty = False
            for inst in bb.instructions:
                si = inst.sync_info
                if si is not None and len(si.on_wait) > max_waits:
                    waits = list(si.on_wait)
                    extra = waits[: len(waits) - max_waits]
                    for w in extra:
                        ev = mybir.InstEventSemaphore(
                            name=f"evs-waitsplit-{n}", ins=[], outs=[]
                        )
                        n += 1
                        ev.engine = inst.engine
                        ev.sync_info = bass_rust.SyncInfo(
                            on_wait=[w], on_update=[]
                        )
                        out.append(ev)
                    si.on_wait = waits[len(waits) - max_waits :]
                    dirty = True
                out.append(inst)
            if dirty:
                bb.instructions = out


def build_program(for_sim: bool = False, n_steps: int = L) -> bass.Bass:
    # for_sim: skip the walrus wait-limit workarounds (post-hoc IR mutations
    # that CoreSim's bookkeeping doesn't understand); semantics identical.
    nc = bass.Bass()

    blob_d = nc.declare_dram_parameter("blob", [NROWS, 512], DT, isOutput=False)
    b32 = blob_d.bitcast(F32)  # [NROWS, 256] f32 view of the same bytes
    # f16 output: per-call upload/launch cost scales with output bytes.
    out_d = nc.declare_dram_parameter("outT", [NCLS, BL], F16, isOutput=True)

    tc_cls = TileContext if for_sim else SplitDrainTileContext
    with tc_cls(nc) as tc:
        with (
            tc.tile_pool(name="singles", bufs=1) as singles,
            tc.tile_pool(name="xchunks", bufs=3) as xpool,
            tc.tile_pool(name="hstate", bufs=2) as hpool,
            tc.tile_pool(name="work", bufs=3) as work,
            tc.tile_pool(name="prz", bufs=1, space="PSUM") as przpool,
            tc.tile_pool(name="pnx", bufs=1, space="PSUM") as pnxpool,
        ):
            # --- constant loads -------------------------------------------
            l1w = singles.tile([DA, G3], DT, tag="l1w")
            hh1w = singles.tile([H, G3], DT, tag="hh1w")
            ih2w = singles.tile([H, G3], DT, tag="ih2w")
            hh2w = singles.tile([H, G3], DT, tag="hh2w")
            sbias = singles.tile([H, 5], F32, tag="sbias")
            fcw = singles.tile([H, NCLS], DT, tag="fcw")
            fcb = singles.tile([NCLS, 1], F32, tag="fcb")
            ident = singles.tile([H, H], DT, tag="ident")
            nc.sync.dma_start(out=ident[:], in_=blob_d[ROW_IDENT : ROW_IDENT + H, 0:H])
            nc.sync.dma_start(out=l1w[:], in_=blob_d[ROW_L1W : ROW_L1W + DA, 0:G3])
            nc.sync.dma_start(out=hh1w[:], in_=blob_d[ROW_HH1 : ROW_HH1 + H, 0:G3])
            nc.sync.dma_start(out=ih2w[:], in_=blob_d[ROW_IH2 : ROW_IH2 + H, 0:G3])
            nc.sync.dma_start(out=hh2w[:], in_=blob_d[ROW_HH2 : ROW_HH2 + H, 0:G3])
            nc.sync.dma_start(out=sbias[:], in_=b32[ROW_BVEC : ROW_BVEC + H, 0:5])
            nc.sync.dma_start(out=fcw[:], in_=blob_d[ROW_FCW : ROW_FCW + H, 0:NCLS])
            nc.sync.dma_start(out=fcb[:], in_=b32[ROW_FCB : ROW_FCB + NCLS, 0:1])

            def halves(ap):
                # [*, BL] access pattern -> its two 512-column halves
                return (ap[:, 0:HB], ap[:, HB:BL])

            def cell(tag, h_prev, x_halves, xw, hw, rz_bias, n_hh_bias,
                     n_ih_bias):
                """One GRU cell step, transposed layout [H parts, BL free].

                h_prev: [H, BL] DT tile or None (t=0 => h=0, recurrent
                matmuls skipped). x_halves: two [K, 512] DT rhs APs for the
                input projection with lhsT xw [K, G3]; hw: [H, G3] recurrent
                lhsT. rz_bias: None (folded into xw) or (r_bias, z_bias)
                APs. Both layers share the przS/pnS/pxS PSUM tiles (the
                pool serializes reuse; layer2 depends on layer1's h anyway).
                Returns the new [H, BL] DT hidden tile.
                """
                prz = przpool.tile([H, 2 * BL], F32, tag="przS")
                pn = pnxpool.tile([H, BL], F32, tag="pnS")
                px = pnxpool.tile([H, BL], F32, tag="pxS")
                hh_halves = halves(h_prev) if h_prev is not None else None
                for i in range(2):
                    # r columns [0:1024], z columns [1024:2048]
                    pr = prz[:, i * HB : (i + 1) * HB]
                    pz = prz[:, BL + i * HB : BL + (i + 1) * HB]
                    nc.tensor.matmul(pr, xw[:, 0:H], x_halves[i],
                                     start=True, stop=h_prev is None)
                    nc.tensor.matmul(pz, xw[:, H : 2 * H], x_halves[i],
                                     start=True, stop=h_prev is None)
                    if h_prev is not None:
                        nc.tensor.matmul(pr, hw[:, 0:H], hh_halves[i],
                                         start=False, stop=True)
                        nc.tensor.matmul(pz, hw[:, H : 2 * H], hh_halves[i],
                                         start=False, stop=True)
                    nc.tensor.matmul(px[:, i * HB : (i + 1) * HB],
                                     xw[:, 2 * H :], x_halves[i],
                                     start=True, stop=False)
                    if h_prev is not None:
                        nc.tensor.matmul(pn[:, i * HB : (i + 1) * HB],
                                         hw[:, 2 * H :], hh_halves[i],
                                         start=True, stop=True)

                rb = dict(bias=rz_bias[0]) if rz_bias else {}
                zb = dict(bias=rz_bias[1]) if rz_bias else {}
                rt = work.tile([H, BL], DT, tag=f"r{tag}")
                nc.scalar.activation(rt[:], prz[:, 0:BL], AF.Sigmoid, **rb)
                zt = work.tile([H, BL], DT, tag=f"z{tag}")
                nc.scalar.activation(zt[:], prz[:, BL:], AF.Sigmoid, **zb)
                r, z = rt[:], zt[:]

                t2 = work.tile([H, BL], DT, tag=f"t2{tag}")
                if h_prev is not None:
                    # t2 = (hn + b_hh_n) * r
                    nc.vector.scalar_tensor_tensor(
                        t2[:], pn[:], n_hh_bias, r, op0=OP.add, op1=OP.mult
                    )
                else:
                    nc.vector.tensor_scalar_mul(t2[:], r, n_hh_bias)
                # px += I.T @ t2 on the PE, then tanh straight off PSUM
                for i in range(2):
                    nc.tensor.matmul(px[:, i * HB : (i + 1) * HB], ident[:],
                                     t2[:, i * HB : (i + 1) * HB],
                                     start=False, stop=True)
                n = work.tile([H, BL], DT, tag=f"n{tag}")
                nb = dict(bias=n_ih_bias) if n_ih_bias is not None else {}
                nc.scalar.activation(n[:], px[:], AF.Tanh, **nb)
                d = work.tile([H, BL], DT, tag=f"d{tag}")
                if h_prev is not None:
                    nc.vector.tensor_sub(d[:], h_prev[:], n[:])
                else:
                    nc.vector.tensor_scalar_mul(d[:], n[:], -1.0)
                e = work.tile([H, BL], DT, tag=f"e{tag}")
                nc.vector.tensor_mul(e[:], z, d[:])
                h_new = hpool.tile([H, BL], DT, tag=f"h{tag}")
                nc.vector.tensor_add(h_new[:], n[:], e[:])
                return h_new

            h1 = None
            h2 = None
            xc = None
            for t in range(n_steps):
                if t % CHUNK == 0:
                    tw = t % L  # wrap: lets timing builds run n_steps > L
                    xc = xpool.tile([DA, CHUNK, 2, HB], DT, tag="xc")
                    nc.sync.dma_start(
                        out=xc[:],
                        in_=blob_d[tw * DA * 2 : (tw + CHUNK) * DA * 2]
                        .rearrange("(t d h) b -> d t h b", t=CHUNK, d=DA),
                    )
                xh = (xc[:, t % CHUNK, 0, :], xc[:, t % CHUNK, 1, :])
                h1 = cell("1", h1, xh, l1w, hh1w, None, sbias[:, 0:1], None)
                h2 = cell("2", h2, halves(h1), ih2w, hh2w,
                          (sbias[:, 1:2], sbias[:, 2:3]), sbias[:, 3:4],
                          sbias[:, 4:5])

            # ---------------- FC head ------------------------------------
            # reuse the pxS PSUM tile (all 8 banks are committed already)
            pfc = pnxpool.tile([H, BL], F32, tag="pxS")
            for i in range(2):
                nc.tensor.matmul(pfc[0:NCLS, i * HB : (i + 1) * HB], fcw[:],
                                 h2[:, i * HB : (i + 1) * HB],
                                 start=True, stop=True)
            outs = work.tile([NCLS, BL], F16, tag="outs")
            nc.scalar.activation(outs[:], pfc[0:NCLS, :], AF.Identity,
                                 bias=fcb[:])
            nc.sync.dma_start(out=out_d[:], in_=outs[:])

    if not for_sim:
        _split_excess_waits(nc)
    return nc


def prep_in_maps(inputs: dict) -> list[dict]:
    """Shard + repack the full-problem numpy inputs into per-core blobs."""
    x = np.ascontiguousarray(np.asarray(inputs["x"], dtype=np.float32))
    w_ih1 = np.asarray(inputs["w_ih1"], np.float32)
    w_hh1 = np.asarray(inputs["w_hh1"], np.float32)
    b_ih1 = np.asarray(inputs["b_ih1"], np.float32)
    b_hh1 = np.asarray(inputs["b_hh1"], np.float32)
    w_ih2 = np.asarray(inputs["w_ih2"], np.float32)
    w_hh2 = np.asarray(inputs["w_hh2"], np.float32)
    b_ih2 = np.asarray(inputs["b_ih2"], np.float32)
    b_hh2 = np.asarray(inputs["b_hh2"], np.float32)
    fc_w = np.asarray(inputs["fc_w"], np.float32)
    fc_b = np.asarray(inputs["fc_b"], np.float32)

    blob = np.zeros((N_CORES, NROWS, 512), np.float16)

    # x: (4096, 2, 1024) -> per-core rows [t, d, half] -> 512-wide halves
    xr = x.reshape(N_CORES, 2, HB, 2, L, D // 2)  # [core, half, b, ch, t, j]
    bxT = blob[:, ROW_XT : ROW_XT + L * DA * 2, :].reshape(
        N_CORES, L, DA, 2, HB
    )
    bxT[:, :, 0 : D // 2] = xr[:, :, :, 0].transpose(0, 3, 4, 1, 2)
    bxT[:, :, D // 2 : D] = xr[:, :, :, 1].transpose(0, 3, 4, 1, 2)
    bxT[:, :, D] = 1.0  # ones row: folds layer-1 biases into the matmul

    # layer-1 combined input-proj weights + bias row.
    # r/z columns carry b_ih1+b_hh1; n columns carry b_ih1 only (b_hh1_n must
    # be applied inside r*(hn+b_hh1_n)).
    l1w = np.empty((DA, G3), np.float32)
    l1w[0:D, :] = w_ih1.T
    bias_row = b_ih1.copy()
    bias_row[0 : 2 * H] += b_hh1[0 : 2 * H]
    l1w[D, :] = bias_row

    bvec = np.stack(
        [
            b_hh1[2 * H : 3 * H],                     # col 0: L1 n-gate hh bias
            (b_ih2 + b_hh2)[0:H],                     # col 1: L2 r bias
            (b_ih2 + b_hh2)[H : 2 * H],               # col 2: L2 z bias
            b_hh2[2 * H : 3 * H],                     # col 3: L2 n-gate hh bias
            b_ih2[2 * H : 3 * H],                     # col 4: L2 n-gate ih bias
        ],
        axis=1,
    ).astype(np.float32)

    blob[:, ROW_L1W : ROW_L1W + DA, 0:G3] = l1w.astype(np.float16)
    blob[:, ROW_HH1 : ROW_HH1 + H, 0:G3] = w_hh1.T.astype(np.float16)
    blob[:, ROW_IH2 : ROW_IH2 + H, 0:G3] = w_ih2.T.astype(np.float16)
    blob[:, ROW_HH2 : ROW_HH2 + H, 0:G3] = w_hh2.T.astype(np.float16)
    blob[:, ROW_FCW : ROW_FCW + H, 0:NCLS] = fc_w.T.astype(np.float16)
    blob[:, ROW_IDENT : ROW_IDENT + H, 0:H] = np.eye(H, dtype=np.float16)
    # f32 regions, stored bit-exact as pairs of f16 slots
    blob[:, ROW_BVEC : ROW_BVEC + H, 0:10] = bvec.view(np.float16)
    blob[:, ROW_FCB : ROW_FCB + NCLS, 0:2] = fc_b[:, None].view(np.float16)

    return [{"blob": blob[c]} for c in range(N_CORES)]


def assemble_output(results: list[dict]) -> np.ndarray:
    # per-core f16 outT [24, BL] -> (4096, 24) f32
    return np.concatenate([r["outT"].T for r in results], axis=0).astype(np.float32)


_NC_CACHE = None
_EXEC_CACHE = None


def bass_io_names(nc: bass.Bass):
    """(in_names, out_names, out_avals) from the program's allocations."""
    import jax

    in_names, out_names, out_avals = [], [], []
    for alloc in nc.m.functions[0].allocations:
        if not isinstance(alloc, mybir.MemoryLocationSet):
            continue
        name = alloc.memorylocations[0].name
        if alloc.kind == "ExternalInput":
            in_names.append(name)
        elif alloc.kind == "ExternalOutput":
            out_avals.append(
                jax.core.ShapedArray(
                    tuple(alloc.tensor_shape), mybir.dt.np(alloc.dtype)
                )
            )
            out_names.append(name)
    return in_names, out_names, out_avals


def build_executor(nc: bass.Bass):
    """Jitted multi-core dispatch of the prebuilt program.

    Leaner than bass_utils.run_bass_kernel_spmd's per-call path: the jit is
    built once and reused, and no donated zero output buffers are passed.
    (With empty input/output aliases the bass_exec lowering allocates fresh
    device buffers for outputs; the zero operands only exist to pre-zero
    outputs for kernels that don't write every element — this kernel writes
    all of outT, so they'd be dead weight uploaded on every call.)
    """
    import jax
    from jax.experimental.shard_map import shard_map
    from jax.sharding import Mesh, PartitionSpec
    from concourse import bass2jax

    bass2jax.install_neuronx_cc_hook()
    assert nc.dbg_addr is None
    partition_name = nc.partition_id_tensor.name if nc.partition_id_tensor else None
    in_names, out_names, out_avals = bass_io_names(nc)
    in_names = [nm for nm in in_names if nm != partition_name]
    all_in_names = list(in_names) + ([partition_name] if partition_name else [])

    def _body(*args):
        operands = list(args)
        if partition_name is not None:
            operands.append(bass2jax.partition_id_tensor())
        outs = bass2jax._bass_exec_p.bind(
            *operands,
            out_avals=tuple(out_avals),
            in_names=tuple(all_in_names),
            out_names=tuple(out_names),
            lowering_input_output_aliases=(),
            sim_require_finite=True,
            sim_require_nnan=True,
            nc=nc,
        )
        return tuple(outs)

    devices = jax.devices()[:N_CORES]
    mesh = Mesh(np.asarray(devices), ("core",))
    sharded = jax.jit(
        shard_map(
            _body,
            mesh=mesh,
            in_specs=(PartitionSpec("core"),) * len(in_names),
            out_specs=(PartitionSpec("core"),) * len(out_names),
            check_rep=False,
        )
    )
    return sharded, in_names, out_names


def kernel(**inputs) -> np.ndarray:
    global _NC_CACHE, _EXEC_CACHE
    if _NC_CACHE is None:
        _NC_CACHE = build_program()
    in_maps = prep_in_maps(inputs)
    try:
        if _EXEC_CACHE is None:
            _EXEC_CACHE = build_executor(_NC_CACHE)
        sharded, in_names, out_names = _EXEC_CACHE
        concat_in = [
            np.concatenate([m[nm] for m in in_maps], axis=0) for nm in in_names
        ]
        outs = [np.asarray(o) for o in sharded(*concat_in)]
        results = [
            {
                nm: outs[i].reshape(
                    N_CORES, outs[i].shape[0] // N_CORES, *outs[i].shape[1:]
                )[c]
                for i, nm in enumerate(out_names)
            }
            for c in range(N_CORES)
        ]
        return assemble_output(results)
    except Exception:
        import traceback

        traceback.print_exc()
        print("kernel: fast dispatch failed; falling back to run_bass_kernel_spmd")
        res = run_bass_kernel_spmd(_NC_CACHE, in_maps, list(range(N_CORES)))
        return assemble_output(res.results)
